# revision 49
# baseline (speedup 1.0000x reference)
"""Trainium2 Bass kernel for nn_Block2_87144886436578.

Reformulation: the reference materializes per-sample jacobians
J[o,m,c,i] = d propagate(x)[o,m] / d x[c,i] but only ever uses two
contractions of J:
  S[o,m,i]  = sum_c J[o,m,c,i]          (-> e_total -> argmin routing)
  Wt[o,m,i] = sum_c x[c,i] J[o,m,c,i]   (-> routed scatter y_masked)
Both are forward-mode JVPs whose input tangents live on a single pixel i:
  v_i = ones over channels at pixel i,  w_i = x[:, i] at pixel i.
So per sample we propagate 2x64 tangents through the ReLU-linearized conv
stack (masks from one forward pass). Batch is data-parallel: sample b ->
core b (8 cores).

Dtypes: S (v-tangent) half runs its convs in f32r (1 cycle/row on PE for
N>=256 vs 4 for fp32); the Wt half's conv inputs are bf16.

Support restriction: tangent i's support after stage s is a (2s+3)-row
window around its row iy, so every mask / conv / accumulate only touches
that window (windows only grow stage to stage, so stale rows outside a
window are always zero).

Emission order interleaves the forward pass with the tangent stages so
the in-order PE queue never stalls on the forward pass's serial
dependency chain.

Layout per half: tangents T [64 part(ch), 64 kk, 10, 10] zero-padded
frames; masked tangents MT [128, ...] where partitions 64-127 hold a
+1-column pre-shift of 0-63 (produced by a flat DMA copy at offset +1),
enabling tap-pair K=128 packing of the 3x3 convs: 6 PE streams instead
of 9. e_total is accumulated into an [8, 8, 64] PSUM tile via one-hot
column lhsT so the argmin reduction runs on 8 partitions.
"""
import os
import numpy as np

F32 = None  # set in _lazy_imports
_CACHE = {}

# S-half conv dtype: "f32r" (4x faster on PE) or "f32" (exact fallback).
S_MODE = os.environ.get('BASS_S_MODE', 'f32r')
# Wt-half conv-input dtype: "bf16" (4x faster) or "f32".
W_MODE = os.environ.get('BASS_W_MODE', 'bf16')


def _lazy_imports():
    global bacc, bass, tile, mybir, F32, BF16, F32R, AX, ALU, ACTF
    import concourse.bacc as bacc
    import concourse.bass as bass
    import concourse.tile as tile
    import concourse.mybir as mybir
    F32 = mybir.dt.float32
    BF16 = mybir.dt.bfloat16
    F32R = mybir.dt.float32r
    AX = mybir.AxisListType
    ALU = mybir.AluOpType
    ACTF = mybir.ActivationFunctionType


ISQRT32 = 0.17677669529663687  # 1/sqrt(32)


def _raw_ap(t_ap, extra_offset, dims):
    """AP on t_ap's tensor: keep partition dim, replace free dims."""
    return bass.AP(tensor=t_ap.tensor, offset=t_ap.offset + extra_offset,
                   ap=[list(t_ap.ap[0])] + [list(d) for d in dims])


def _win(lo, hi, min_rows=0):
    lo, hi = max(0, lo), min(8, hi)
    while hi - lo < min_rows:
        if hi < 8:
            hi += 1
        else:
            lo -= 1
    return lo, hi


def build_nc():
    _lazy_imports()
    nc = bacc.Bacc("TRN2", target_bir_lowering=False, debug=True)

    # f32r-consumed tensors are declared float32r end-to-end (the BIR
    # verifier requires every producer of f32r-matmul operands to round
    # to f32r); the forward pass keeps separate fp32 weight copies.
    SDT = F32R if S_MODE == 'f32r' else F32
    s_min_rows = 4 if S_MODE == 'f32r' else 0
    WDT = {'bf16': BF16, 'f32': F32}[W_MODE]

    # ---- DRAM I/O (per-core; weights replicated across cores) ----
    d_x = nc.dram_tensor("x", [64, 64], F32, kind="ExternalInput")
    d_w1T = nc.dram_tensor("w1T", [64, 9, 64], F32, kind="ExternalInput")
    d_pk64 = nc.dram_tensor("pk64", [64, 161], F32, kind="ExternalInput")
    d_pk32 = nc.dram_tensor("pk32", [32, 577], F32, kind="ExternalInput")
    d_r0w1T = nc.dram_tensor("r0w1T", [64, 9, 32], F32, kind="ExternalInput")
    d_r0w2T = nc.dram_tensor("r0w2T", [64, 64], F32, kind="ExternalInput")
    d_r1w1T = nc.dram_tensor("r1w1T", [64, 9, 32], F32, kind="ExternalInput")
    d_r1w2T = nc.dram_tensor("r1w2T", [64, 64], F32, kind="ExternalInput")
    d_w1s0 = nc.dram_tensor("w1s0", [64, 9, 96], SDT, kind="ExternalInput")
    d_w1s1 = nc.dram_tensor("w1s1", [64, 9, 96], SDT, kind="ExternalInput")
    d_w1p0 = nc.dram_tensor("w1p0", [128, 3, 96], SDT, kind="ExternalInput")
    d_w1p1 = nc.dram_tensor("w1p1", [128, 3, 96], SDT, kind="ExternalInput")
    d_w2r0 = nc.dram_tensor("w2r0", [64, 2, 64], SDT, kind="ExternalInput")
    d_w2r1 = nc.dram_tensor("w2r1", [64, 2, 64], SDT, kind="ExternalInput")
    d_oh8r = nc.dram_tensor("oh8r", [64, 256], SDT, kind="ExternalInput")
    d_pat = nc.dram_tensor("patterns", [128, 4, 32], F32, kind="ExternalInput")
    d_patTr = nc.dram_tensor("patTr", [32, 512], SDT, kind="ExternalInput")
    d_ohrep = nc.dram_tensor("ohrep", [8, 8, 64], F32, kind="ExternalInput")
    d_zer = nc.dram_tensor("zer", [64, 3200], F32, kind="ExternalInput")
    d_out = nc.dram_tensor("out", [32, 64], F32, kind="ExternalOutput")

    with tile.TileContext(nc) as tc:
        with (
            tc.tile_pool(name="big", bufs=1) as big,
            tc.tile_pool(name="tmp", bufs=4) as tmp,
            tc.tile_pool(name="psum", bufs=3, space="PSUM") as ps,
            tc.tile_pool(name="psumj", bufs=4, space="PSUM") as psj,
            tc.tile_pool(name="psume", bufs=1, space="PSUM") as pse,
        ):
            _ps_n = [0]

            def pst(shape):
                _ps_n[0] += 1
                return ps.tile(shape, F32, tag="ps", name=f"ps{_ps_n[0]}")

            # ---- persistent SBUF ----
            T32 = big.tile([64, 64, 10, 10], F32, tag="T32")
            MT32 = big.tile([128, 64, 10, 10], SDT, tag="MT32")
            MH32 = big.tile([64, 4, 8, 64], SDT, tag="MH32")  # [2par*h, j, kk8, pix]
            T16 = big.tile([64, 64, 10, 10], F32, tag="T16")
            MT16 = big.tile([128, 64, 10, 10], WDT, tag="MT16")
            MH16 = big.tile([64, 4, 8, 64], WDT, tag="MH16")

            w1T = big.tile([64, 9, 64], F32, tag="w1T")
            pk64 = big.tile([64, 161], F32, tag="pk64")
            pk32 = big.tile([32, 577], F32, tag="pk32")
            b1 = pk64[:, 0:1]
            c2wT = pk64[:, 1:33]
            ident = pk64[:, 33:97]
            b2 = pk32[:, 0:1]
            c2w_oc = pk32[:, 1:65]
            patT = pk32[:, 65:577]
            r0w1T = big.tile([64, 9, 32], F32, tag="r0w1T")
            r1w1T = big.tile([64, 9, 32], F32, tag="r1w1T")
            r0w2T = big.tile([64, 64], F32, tag="r0w2T")   # parity-dup rows
            r1w2T = big.tile([64, 64], F32, tag="r1w2T")
            # conv weights in par-padded layouts: singles [64, 9, 96]
            # (cols w|0|w, par slice at 32*par), packed [128, 3, 96],
            # w2 [64(2par*h), 2(par), 64] -- everything partition-base 0
            w1s0 = big.tile([64, 9, 96], SDT, tag="w1s0")
            w1s1 = big.tile([64, 9, 96], SDT, tag="w1s1")
            w1p0 = big.tile([128, 3, 96], SDT, tag="w1p0")
            w1p1 = big.tile([128, 3, 96], SDT, tag="w1p1")
            w2r0 = big.tile([64, 2, 64], SDT, tag="w2r0")
            w2r1 = big.tile([64, 2, 64], SDT, tag="w2r1")
            oh8r = big.tile([64, 256], SDT, tag="oh8r")
            w1s0b = big.tile([64, 9, 96], WDT, tag="w1s0b")
            w1s1b = big.tile([64, 9, 96], WDT, tag="w1s1b")
            w1p0b = big.tile([128, 3, 96], WDT, tag="w1p0b")
            w1p1b = big.tile([128, 3, 96], WDT, tag="w1p1b")
            w2r0b = big.tile([64, 2, 64], WDT, tag="w2r0b")
            w2r1b = big.tile([64, 2, 64], WDT, tag="w2r1b")
            pat = big.tile([128, 4, 32], F32, tag="pat")
            ohrep = big.tile([8, 8, 64], F32, tag="ohrep")
            ohrep_b = big.tile([8, 8, 64], BF16, tag="ohrep_b")
            ones64 = big.tile([64, 64], F32, tag="ones64")
            patTr = big.tile([32, 512], SDT, tag="patTr")
            ymr = big.tile([32, 64], SDT, tag="ymr")
            ohf8 = big.tile([8, 8, 64], BF16, tag="ohf8")
            mn8 = big.tile([8, 8, 1], F32, tag="mn8")

            x_pad = big.tile([64, 10, 10], F32, tag="x_pad")
            a_pad = big.tile([64, 10, 10], F32, tag="a_pad")
            m1a = big.tile([64, 64], F32, tag="m1a")
            m2a = big.tile([64, 64], F32, tag="m2a")
            m3 = big.tile([64, 64], F32, tag="m3")
            m1b = big.tile([64, 64], F32, tag="m1b")   # parity-dup at +32
            m2b = big.tile([64, 64], F32, tag="m2b")
            y1 = big.tile([64, 64], F32, tag="y1")
            y2 = big.tile([64, 64], F32, tag="y2")
            y3 = big.tile([64, 64], F32, tag="y3")
            y4 = big.tile([64, 64], F32, tag="y4")
            yout = big.tile([32, 64], F32, tag="yout")
            r_sb = big.tile([32, 64], F32, tag="r_sb")
            V = big.tile([64, 64], F32, tag="V")       # (c2w^T r) * m3
            P1 = big.tile([64, 512], F32, tag="P1")
            P2 = big.tile([64, 512], F32, tag="P2")
            ym = big.tile([32, 64, 1], F32, tag="ym")
            Gq = big.tile([64, 64, 8], F32, tag="Gq")
            G = big.tile([64, 64, 1], F32, tag="G")
            out_sb = big.tile([32, 64], F32, tag="out_sb")
            prodE = big.tile([64, 64, 64], SDT, tag="prodE")
            prodW = big.tile([64, 64, 64], F32, tag="prodW")

            # ---- loads: early-needed first per queue; Pool does no DMA ----
            sdma = nc.sync.dma_start
            adma = nc.scalar.dma_start
            sdma(out=x_pad[:, 1:9, 1:9],
                 in_=d_x[:].rearrange("c (y x) -> c y x", y=8))
            sdma(out=w1T[:], in_=d_w1T[:])
            sdma(out=pk64[:], in_=d_pk64[:])
            sdma(out=r0w1T[:], in_=d_r0w1T[:])
            sdma(out=w1s0[:], in_=d_w1s0[:])
            sdma(out=w1p0[:], in_=d_w1p0[:])
            sdma(out=r0w2T[:], in_=d_r0w2T[:])
            sdma(out=w2r0[:], in_=d_w2r0[:])
            sdma(out=pk32[:], in_=d_pk32[:])
            sdma(out=r1w1T[:], in_=d_r1w1T[:])
            sdma(out=w1s1[:], in_=d_w1s1[:])
            sdma(out=w1p1[:], in_=d_w1p1[:])
            sdma(out=r1w2T[:], in_=d_r1w2T[:])
            sdma(out=w2r1[:], in_=d_w2r1[:])
            sdma(out=oh8r[:], in_=d_oh8r[:])
            sdma(out=pat[:], in_=d_pat[:])
            sdma(out=ohrep[:], in_=d_ohrep[:])

            # ---- memsets: T frames zeroed early on Pool+DVE; Act's queue
            # stays clear for the forward-pass relus ----
            nc.vector.memset(x_pad[:, :, 0:1], 0.0)
            nc.vector.memset(x_pad[:, :, 9:10], 0.0)
            nc.vector.memset(x_pad[:, 0, 1:9], 0.0)
            nc.vector.memset(x_pad[:, 9, 1:9], 0.0)
            nc.vector.memset(a_pad[:], 0.0)
            nc.vector.memset(ones64[:], 1.0)
            nc.gpsimd.memset(T32[:, 0:32, :, :], 0.0)
            nc.scalar.memzero(T32[:, 32:64, :, :])
            nc.gpsimd.memset(T16[:, 0:32, :, :], 0.0)
            nc.vector.memset(T16[:, 32:64, :, :], 0.0)
            # MT lower borders (upper halves are rewritten by the shift-copy)
            nc.scalar.memzero(MT16[0:64, :, 0, :])
            nc.scalar.memzero(MT16[0:64, :, 9, :])
            nc.gpsimd.memset(MT16[0:64, :, 1:9, 0], 0.0)
            nc.gpsimd.memset(MT16[0:64, :, 1:9, 9], 0.0)
            for reg in ((slice(None), 0, slice(None)),
                        (slice(None), 9, slice(None)),
                        (slice(None), slice(1, 9), 0),
                        (slice(None), slice(1, 9), 9)):
                nc.vector.tensor_tensor(
                    out=MT32[(slice(0, 64),) + reg],
                    in0=T32[(slice(None),) + reg],
                    in1=T32[(slice(None),) + reg], op=ALU.mult)
            nc.vector.tensor_copy(w1s0b[:], w1s0[:])
            nc.vector.tensor_copy(w1p0b[:], w1p0[:])
            nc.vector.tensor_copy(w2r0b[:], w2r0[:])
            nc.gpsimd.tensor_copy(w1s1b[:], w1s1[:])
            nc.gpsimd.tensor_copy(w1p1b[:], w1p1[:])
            nc.gpsimd.tensor_copy(w2r1b[:], w2r1[:])

            TAPS = [(ky, kx) for ky in range(3) for kx in range(3)]

            def conv9(out_ps, wT_d, src_pad, M):
                for t, (ky, kx) in enumerate(TAPS):
                    nc.tensor.matmul(
                        out_ps, wT_d[:, t, :M],
                        src_pad[:, ky:ky + 8, kx:kx + 8],
                        start=(t == 0), stop=(t == 8))

            # ================= tangent init =================
            # T[p, kk=(iy,ix), iy+ky, ix+kx] = VW[p, (2-ky,2-kx), kk],
            # scattered straight from PSUM. Also warms up the PE pstate
            # before the forward pass's serial chain.
            for (ky, kx) in TAPS:
                t_src = (2 - ky) * 3 + (2 - kx)
                vwp = pst([64, 64])
                nc.tensor.matmul(vwp[:], w1T[:, t_src, :], ones64[:],
                                 start=True, stop=True)
                nc.vector.tensor_copy(
                    _raw_ap(T32[:], ky * 10 + kx, [[810, 8], [101, 8]]),
                    _raw_ap(vwp[:], 0, [[8, 8], [1, 8]]))
                vwq = pst([64, 64])
                nc.tensor.matmul(vwq[:], w1T[:, t_src, :],
                                 x_pad[:, 1:9, 1:9], start=True, stop=True)
                nc.vector.tensor_copy(
                    _raw_ap(T16[:], ky * 10 + kx, [[810, 8], [101, 8]]),
                    _raw_ap(vwq[:], 0, [[8, 8], [1, 8]]))

            # ================= forward head =================
            y1p = pst([64, 64])
            conv9(y1p[:], w1T, x_pad, 64)
            nc.vector.tensor_scalar(out=y1[:], in0=y1p[:], scalar1=b1,
                                    scalar2=None, op0=ALU.add)
            nc.vector.tensor_scalar(out=m1a[:], in0=y1[:], scalar1=0.0,
                                    scalar2=None, op0=ALU.is_gt)
            nc.scalar.activation(
                out=a_pad[:, 1:9, 1:9],
                in_=y1[:].rearrange("c (y x) -> c y x", y=8), func=ACTF.Relu)

            def fwd_block(w1T_d, w2T_d, mb, ma_next, y_in, y_out):
                hp = pst([32, 64])
                conv9(hp[:], w1T_d, a_pad, 32)
                nc.vector.tensor_scalar(out=mb[0:32, :], in0=hp[:], scalar1=0.0,
                                        scalar2=None, op0=ALU.is_gt)
                sdma(out=mb[32:64, :], in_=mb[0:32, :])
                bh = tmp.tile([32, 64], F32, tag="bh")
                nc.vector.tensor_scalar_max(bh[:], hp[:], 0.0)
                up = pst([64, 64])
                nc.tensor.matmul(up[:], w2T_d[0:32, 0:64], bh[:],
                                 start=True, stop=True)
                nc.vector.tensor_tensor(out=y_out[:], in0=y_in[:], in1=up[:],
                                        op=ALU.add)
                nc.vector.tensor_scalar(out=ma_next[:], in0=y_out[:],
                                        scalar1=0.0, scalar2=None, op0=ALU.is_gt)

            # ================= tangent stage phases =================
            # cfg = (Tt, MTt, MHt, w1s, w1p, w2T, cast, dma_q, acc_eng, minr)
            def phase_mask_j(s, cfgs, ma, j):
                    for cf in cfgs:
                        Tt, MTt, dq, meng = cf[0], cf[1], cf[7], cf[11]
                        o_lo, o_hi = _win(2 * j - s - 1, 2 * j + s + 3, cf[9])
                        m_lo, m_hi = max(0, o_lo - 1), min(8, o_hi + 1)
                        meng.tensor_tensor(
                            out=MTt[0:64, 16 * j:16 * j + 16,
                                    1 + m_lo:1 + m_hi, 1:9],
                            in0=Tt[:, 16 * j:16 * j + 16,
                                   1 + m_lo:1 + m_hi, 1:9],
                            in1=ma[:, 8 * m_lo:8 * m_hi].rearrange(
                                "p (k y x) -> p k y x", k=1, y=m_hi - m_lo)
                                .broadcast_to((64, 16, m_hi - m_lo, 8)),
                            op=ALU.mult)
                        # upper half = +1-flat-shift of the lower via DMA
                        dq(out=_raw_ap(MTt[64:128, :, :, :], 1600 * j,
                                       [[1, 1599]]),
                           in_=_raw_ap(MTt[0:64, :, :, :], 1600 * j + 1,
                                       [[1, 1599]]))

            def phase_mask(s, cfgs, ma):
                for j in range(4):
                    phase_mask_j(s, cfgs, ma, j)

            def phase_conva(s, cfgs):
                for j in range(4):
                    for cf in cfgs:
                        MTt, w1s_t, w1p_t, cast, minr = (
                            cf[1], cf[3], cf[4], cf[6], cf[9])
                        o_lo, o_hi = _win(2 * j - s - 1, 2 * j + s + 3, minr)
                        rows = o_hi - o_lo
                        _ps_n[0] += 1
                        pj = psj.tile([64, 8, rows, 8], F32, tag="pj",
                                      name=f"pj{_ps_n[0]}")
                        for par in range(2):
                            qq = 2 * j + par
                            # 3 single (taps (ky,2), K=64) + 3 packed
                            # (taps (ky,0)+(ky,1), K=128) streams; the
                            # par-padded lhsT slice routes par outputs to
                            # partition halves of one accumulation region
                            for ky in range(3):
                                nc.tensor.matmul(
                                    pj[:],
                                    cast(w1s_t[:, 3 * ky + 2,
                                               32 * par:32 * par + 64]),
                                    cast(MTt[0:64, 8 * qq:8 * qq + 8,
                                             ky + o_lo:ky + o_hi, 2:10]),
                                    start=(par == 0 and ky == 0), stop=False)
                                nc.tensor.matmul(
                                    pj[:],
                                    cast(w1p_t[:, ky,
                                               32 * par:32 * par + 64]),
                                    cast(MTt[0:128, 8 * qq:8 * qq + 8,
                                             ky + o_lo:ky + o_hi, 0:8]),
                                    start=False, stop=(par == 1 and ky == 2))
                        cf[10].append(pj)

            def phase_mh_convb_acc(s, cfgs, mb, after_j=None):
                for j in range(4):
                    for cf in cfgs:
                        MHt, minr = cf[2], cf[9]
                        o_lo, o_hi = _win(2 * j - s - 1, 2 * j + s + 3, minr)
                        rows = o_hi - o_lo
                        pj = cf[10][j]
                        nc.vector.tensor_tensor(
                            out=MHt[:, j, :, 8 * o_lo:8 * o_hi],
                            in0=pj[:].rearrange("p k r x -> p k (r x)"),
                            in1=mb[:, 8 * o_lo:8 * o_hi].rearrange(
                                "p (k m) -> p k m", k=1)
                                .broadcast_to((64, 8, 8 * rows)),
                            op=ALU.mult)
                for qq in range(8):
                    if after_j is not None and qq >= 2 and qq % 2 == 0:
                        after_j(qq // 2 - 1)
                    j, par = qq // 2, qq % 2
                    for cf in cfgs:
                        (Tt, MTt, MHt, w1s_t, w1p_t, w2T_t,
                         cast, dq, aeng, minr, _pjs, _meng) = cf
                        q_lo, q_hi = _win(qq - s - 1, qq + s + 2, minr)
                        rows = q_hi - q_lo
                        uq = pst([64, 8, rows, 8])
                        nc.tensor.matmul(
                            uq[:].rearrange("p k r x -> p k (r x)"),
                            cast(w2T_t[:, par, :]),
                            cast(MHt[:, j, :, 8 * q_lo:8 * q_hi]),
                            start=True, stop=True)
                        if aeng is nc.gpsimd:
                            # GpSimd can't read PSUM: stage uq through SBUF
                            # on Act, then add on Pool (keeps DVE free)
                            uqsb = tmp.tile([64, 8, rows, 8], F32, tag="uqsb")
                            nc.scalar.copy(uqsb[:], uq[:])
                            acc_in = uqsb
                        else:
                            acc_in = uq
                        aeng.tensor_tensor(
                            out=Tt[:, 8 * qq:8 * qq + 8,
                                   1 + q_lo:1 + q_hi, 1:9],
                            in0=Tt[:, 8 * qq:8 * qq + 8,
                                   1 + q_lo:1 + q_hi, 1:9],
                            in1=acc_in[:],
                            op=ALU.add)
                if after_j is not None:
                    after_j(3)

            def w_cast(ap):
                return ap

            cfgs1 = [
                [T32, MT32, MH32, w1s0, w1p0, w2r0, w_cast,
                 sdma, nc.vector, s_min_rows, [], nc.vector],
                [T16, MT16, MH16, w1s0b, w1p0b, w2r0b, w_cast,
                 adma, nc.vector, 0, [], nc.gpsimd],
            ]
            cfgs2 = [
                [T32, MT32, MH32, w1s1, w1p1, w2r1, w_cast,
                 sdma, nc.vector, s_min_rows, [], nc.vector],
                [T16, MT16, MH16, w1s1b, w1p1b, w2r1b, w_cast,
                 adma, nc.vector, 0, [], nc.gpsimd],
            ]

            # ---- interleaved emission: fwd blocks fill PE gaps ----
            fwd_block(r0w1T, r0w2T, m1b, m2a, y1, y2)
            nc.scalar.activation(
                out=a_pad[:, 1:9, 1:9],
                in_=y2[:].rearrange("c (y x) -> c y x", y=8), func=ACTF.Relu)
            phase_mask(1, cfgs1, m1a)
            phase_conva(1, cfgs1)
            fwd_block(r1w1T, r1w2T, m2b, m3, y2, y3)
            nc.scalar.activation(out=y4[:], in_=y3[:], func=ACTF.Relu)
            yop = pst([32, 64])
            nc.tensor.matmul(yop[:], c2wT, y4[:], start=True, stop=True)
            nc.vector.tensor_scalar(out=yout[:], in0=yop[:], scalar1=b2,
                                    scalar2=None, op0=ALU.add)
            phase_mh_convb_acc(
                1, cfgs1, m1b,
                after_j=lambda j: phase_mask_j(2, cfgs2, m2a, j))

            # ================= hopfield helper =================
            def hopfield(y_ap, P, fast):
                lg = pst([64, 512])
                nc.tensor.matmul(lg[:], y_ap, patT, start=True, stop=True)
                ssum = tmp.tile([64, 1], F32, tag="ssum")
                # logits are tame (|lg|/sqrt(32) < 40): skip max-stabilization
                nc.scalar.activation(out=P[:], in_=lg[:], func=ACTF.Exp,
                                     bias=0.0, scale=ISQRT32, accum_out=ssum[:])
                rs = tmp.tile([64, 1], F32, tag="rs")
                nc.vector.reciprocal(rs[:], ssum[:])
                nc.scalar.mul(P[:], P[:], rs[:])
                yq = pse.tile([32, 64], F32, tag="et8", name=f"yq{_ps_n[0]}")
                for qc in range(4):
                    ptp = pst([128, 64])
                    pt = tmp.tile([128, 64], F32, tag="pt")
                    nc.tensor.transpose(ptp[:], P[:, 128 * qc:128 * (qc + 1)],
                                        ident)
                    if qc % 2 == 0:
                        nc.vector.tensor_copy(pt[:], ptp[:])
                    else:
                        nc.scalar.copy(pt[:], ptp[:])
                    nc.tensor.matmul(yq[:], pat[:, qc, :], pt[:],
                                     start=(qc == 0), stop=(qc == 3))
                return yq

            yq1 = hopfield(yout[:], P1, False)
            nc.vector.tensor_tensor(out=r_sb[:], in0=yout[:], in1=yq1[:],
                                    op=ALU.subtract)
            rps = pst([64, 64])
            nc.tensor.matmul(rps[:], c2w_oc, r_sb[:], start=True, stop=True)
            nc.vector.tensor_tensor(out=V[:], in0=rps[:], in1=m3[:],
                                    op=ALU.mult)
            phase_conva(2, cfgs2)

            # ================= C2 + routing + scatter =================
            # e_total[i,m] = sum_c V[c,m] * T32[c,i,m]; the prodE/et8
            # chunks are emitted inside stage-2's acc phase as each T32
            # chunk finalizes
            et8 = pse.tile([32, 8, 64], F32, tag="et8", name="et8")

            def emit_prode(j):
                for qq in (2 * j, 2 * j + 1):
                    peng = nc.vector if qq % 2 == 0 else nc.gpsimd
                    peng.tensor_tensor(
                        out=prodE[:, 8 * qq:8 * qq + 8, :]
                            .rearrange("p k (y x) -> p k y x", y=8),
                        in0=T32[:, 8 * qq:8 * qq + 8, 1:9, 1:9],
                        in1=V[:].rearrange("p (k y x) -> p k y x", k=1, y=8)
                            .broadcast_to((64, 8, 8, 8)),
                        op=ALU.mult)
                    # partition-sum lands on psum partition qq via one-hot
                    nc.tensor.matmul(
                        et8[:].rearrange("p k m -> p (k m)"),
                        oh8r[:, 32 * qq:32 * qq + 32],
                        prodE[:, 8 * qq:8 * qq + 8, :]
                            .rearrange("p k m -> p (k m)"),
                        start=(qq == 0), stop=(qq == 7))

            phase_mh_convb_acc(2, cfgs2, m2b, after_j=emit_prode)
            nc.vector.tensor_reduce(out=mn8[:], in_=et8[0:8, :, :], axis=AX.X,
                                    op=ALU.min)
            nc.vector.tensor_tensor(out=ohf8[:], in0=et8[0:8, :, :],
                                    in1=mn8[:].broadcast_to((8, 8, 64)),
                                    op=ALU.is_equal)
            nc.gpsimd.tensor_copy(ohrep_b[:], ohrep[:])
            # scatter: prodW[c, m, i] = T16[c, i, m] * onehot[i, m], then
            # per-chunk partial i-sums Gq -> G
            for qq in range(8):
                rep = pst([64, 8, 64])
                nc.tensor.matmul(
                    rep[:], ohrep_b[:, qq, :],
                    ohf8[:].rearrange("p k m -> p (k m)"),
                    start=True, stop=True)
                dst = _raw_ap(prodW[:], 8 * qq, [[1, 8], [512, 8], [64, 8]])
                nc.vector.tensor_tensor(
                    out=dst,
                    in0=T16[:, 8 * qq:8 * qq + 8, 1:9, 1:9],
                    in1=rep[:].rearrange("p k (y x) -> p k y x", y=8),
                    op=ALU.mult)
                nc.vector.tensor_reduce(out=Gq[:, :, qq:qq + 1],
                                        in_=prodW[:, :, 8 * qq:8 * qq + 8],
                                        axis=AX.X, op=ALU.add)
            nc.vector.tensor_reduce(out=G[:], in_=Gq[:], axis=AX.X, op=ALU.add)
            nc.vector.tensor_tensor(
                out=G[:, :, 0], in0=G[:, :, 0], in1=m3[:], op=ALU.mult)
            ymp = pst([32, 64])
            for m0 in (0, 32):
                nc.tensor.matmul(ymp[:, m0:m0 + 32], c2wT,
                                 G[:, m0:m0 + 32, 0], start=True, stop=True)
            nc.vector.tensor_copy(ym[:, :, 0], ymp[:])

            yq2 = hopfield(ym[:, :, 0], P2, True)
            nc.vector.tensor_copy(out_sb[:], yq2[:])
            sdma(out=d_out[:], in_=out_sb[:])

    nc.compile()
    return nc


def _prep_weights(inputs):
    f = np.float32
    w1 = np.asarray(inputs['conv1_w'], f)
    w1t = w1.transpose(2, 3, 1, 0).reshape(9, 64, 64)         # [tap, c, o]
    r0 = np.asarray(inputs['res0_w1'], f).transpose(2, 3, 1, 0).reshape(9, 64, 32)
    r1 = np.asarray(inputs['res1_w1'], f).transpose(2, 3, 1, 0).reshape(9, 64, 32)
    r0w2 = np.asarray(inputs['res0_w2'], f)[:, :, 0, 0].T      # [32, 64]
    r1w2 = np.asarray(inputs['res1_w2'], f)[:, :, 0, 0].T
    pats = np.asarray(inputs['patterns'], f)

    def pack_p(r):   # [128, 3, 32]: parts 0-63 taps (ky,0), 64-127 taps (ky,1)
        return np.concatenate([r[[0, 3, 6]].transpose(1, 0, 2),
                               r[[1, 4, 7]].transpose(1, 0, 2)], axis=0)

    def dup2(w2):    # [64, 64]: parity-dup rows
        return np.concatenate([w2, w2], axis=0)

    def pad96(w):    # [P, T, 32] -> [P, T, 96] cols (w | 0 | w)
        P, T, _ = w.shape
        z = np.zeros((P, T, 96), f)
        z[:, :, 0:32] = w
        z[:, :, 64:96] = w
        return np.ascontiguousarray(z)

    def w2pad(w2):   # [32, 64] -> [64(2par*h), 2(par), 64]: par-selecting
        z = np.zeros((64, 2, 64), f)
        z[0:32, 0, :] = w2
        z[32:64, 1, :] = w2
        return np.ascontiguousarray(z)

    c = np.ascontiguousarray
    # pk64 = [b1 | c2w^T | I64 | oh8], pk32 = [b2 | c2w]
    pk64 = np.concatenate([
        np.asarray(inputs['conv1_b'], f).reshape(64, 1),
        np.asarray(inputs['conv2_w'], f)[:, :, 0, 0].T,
        np.eye(64, dtype=f),
        np.broadcast_to(np.eye(8, dtype=f), (64, 8, 8)).reshape(64, 64),
    ], axis=1)
    pk32 = np.concatenate([
        np.asarray(inputs['conv2_b'], f).reshape(32, 1),
        np.asarray(inputs['conv2_w'], f)[:, :, 0, 0],
        pats.T,
    ], axis=1)
    base = {
        'w1T': c(w1t.transpose(1, 0, 2)),
        'pk64': c(pk64),
        'pk32': c(pk32),
        'r0w1T': c(r0.transpose(1, 0, 2)),
        'w1s0': pad96(r0.transpose(1, 0, 2)),
        'w1s1': pad96(r1.transpose(1, 0, 2)),
        'w1p0': pad96(pack_p(r0)),
        'w1p1': pad96(pack_p(r1)),
        'w2r0': w2pad(r0w2),
        'w2r1': w2pad(r1w2),
        'oh8r': c(np.broadcast_to(np.concatenate(
            [np.eye(8, dtype=f), np.zeros((8, 24), f)], axis=1),
            (64, 8, 32)).reshape(64, 256)),
        'r0w2T': c(dup2(r0w2)),
        'r1w1T': c(r1.transpose(1, 0, 2)),
        'r1w2T': c(dup2(r1w2)),
        'patterns': c(pats.reshape(4, 128, 32).transpose(1, 0, 2)),
        'patternsT': c(pats.T),
        'patTr': c(pats.T),
        'zer': np.zeros((64, 3200), f),
        'ohrep': c(np.broadcast_to(np.eye(8, dtype=f)[:, :, None], (8, 8, 64))),
    }
    return base


def make_in_maps(inputs):
    x = np.asarray(inputs['x'], np.float32)
    base = _prep_weights(inputs)
    return [dict(base, x=np.ascontiguousarray(x[b].reshape(64, 64)))
            for b in range(8)]


def kernel(**inputs):
    _lazy_imports()
    from concourse.bass_utils import run_bass_kernel_spmd
    if 'nc' not in _CACHE:
        _CACHE['nc'] = build_nc()
    nc = _CACHE['nc']
    in_maps = make_in_maps(inputs)
    res = run_bass_kernel_spmd(nc, in_maps, list(range(8)))
    _CACHE['last_result'] = res
    out = np.stack([res.results[b]['out'].reshape(32, 8, 8) for b in range(8)])
    return out.astype(np.float32)


# revision 52
# speedup vs baseline: 1.0547x; 1.0547x over previous
"""Trainium2 Bass kernel for nn_Block2_87144886436578.

Reformulation: the reference materializes per-sample jacobians
J[o,m,c,i] = d propagate(x)[o,m] / d x[c,i] but only ever uses two
contractions of J:
  S[o,m,i]  = sum_c J[o,m,c,i]          (-> e_total -> argmin routing)
  Wt[o,m,i] = sum_c x[c,i] J[o,m,c,i]   (-> routed scatter y_masked)
Both are forward-mode JVPs whose input tangents live on a single pixel i:
  v_i = ones over channels at pixel i,  w_i = x[:, i] at pixel i.
So per sample we propagate 2x64 tangents through the ReLU-linearized conv
stack (masks from one forward pass). Batch is data-parallel: sample b ->
core b (8 cores).

Dtypes: S (v-tangent) half runs its convs in f32r (1 cycle/row on PE for
N>=256 vs 4 for fp32); the Wt half's conv inputs are bf16.

Support restriction: tangent i's support after stage s is a (2s+3)-row
window around its row iy, so every mask / conv / accumulate only touches
that window (windows only grow stage to stage, so stale rows outside a
window are always zero).

Emission order interleaves the forward pass with the tangent stages so
the in-order PE queue never stalls on the forward pass's serial
dependency chain.

Layout per half: tangents T [64 part(ch), 64 kk, 10, 10] zero-padded
frames; masked tangents MT [128, ...] where partitions 64-127 hold a
+1-column pre-shift of 0-63 (produced by a flat DMA copy at offset +1),
enabling tap-pair K=128 packing of the 3x3 convs: 6 PE streams instead
of 9. e_total is accumulated into an [8, 8, 64] PSUM tile via one-hot
column lhsT so the argmin reduction runs on 8 partitions.
"""
import os
import numpy as np

F32 = None  # set in _lazy_imports
_CACHE = {}

# S-half conv dtype: "f32r" (4x faster on PE) or "f32" (exact fallback).
S_MODE = os.environ.get('BASS_S_MODE', 'f32r')
# Wt-half conv-input dtype: "bf16" (4x faster) or "f32".
W_MODE = os.environ.get('BASS_W_MODE', 'bf16')


def _lazy_imports():
    global bacc, bass, tile, mybir, F32, BF16, F32R, AX, ALU, ACTF
    import concourse.bacc as bacc
    import concourse.bass as bass
    import concourse.tile as tile
    import concourse.mybir as mybir
    F32 = mybir.dt.float32
    BF16 = mybir.dt.bfloat16
    F32R = mybir.dt.float32r
    AX = mybir.AxisListType
    ALU = mybir.AluOpType
    ACTF = mybir.ActivationFunctionType


ISQRT32 = 0.17677669529663687  # 1/sqrt(32)


def _raw_ap(t_ap, extra_offset, dims):
    """AP on t_ap's tensor: keep partition dim, replace free dims."""
    return bass.AP(tensor=t_ap.tensor, offset=t_ap.offset + extra_offset,
                   ap=[list(t_ap.ap[0])] + [list(d) for d in dims])


def _win(lo, hi, min_rows=0):
    lo, hi = max(0, lo), min(8, hi)
    while hi - lo < min_rows:
        if hi < 8:
            hi += 1
        else:
            lo -= 1
    return lo, hi


def build_nc():
    _lazy_imports()
    nc = bacc.Bacc("TRN2", target_bir_lowering=False, debug=True)

    # f32r-consumed tensors are declared float32r end-to-end (the BIR
    # verifier requires every producer of f32r-matmul operands to round
    # to f32r); the forward pass keeps separate fp32 weight copies.
    SDT = F32R if S_MODE == 'f32r' else F32
    s_min_rows = 4 if S_MODE == 'f32r' else 0
    WDT = {'bf16': BF16, 'f32': F32}[W_MODE]

    # ---- DRAM I/O (per-core; weights replicated across cores) ----
    d_x = nc.dram_tensor("x", [64, 64], F32, kind="ExternalInput")
    d_w1T = nc.dram_tensor("w1T", [64, 9, 64], F32, kind="ExternalInput")
    d_pk64 = nc.dram_tensor("pk64", [64, 161], F32, kind="ExternalInput")
    d_pk32 = nc.dram_tensor("pk32", [32, 577], F32, kind="ExternalInput")
    d_r0w1T = nc.dram_tensor("r0w1T", [64, 9, 32], F32, kind="ExternalInput")
    d_r0w2T = nc.dram_tensor("r0w2T", [64, 64], F32, kind="ExternalInput")
    d_r1w1T = nc.dram_tensor("r1w1T", [64, 9, 32], F32, kind="ExternalInput")
    d_r1w2T = nc.dram_tensor("r1w2T", [64, 64], F32, kind="ExternalInput")
    d_w1s0 = nc.dram_tensor("w1s0", [64, 9, 96], SDT, kind="ExternalInput")
    d_w1s1 = nc.dram_tensor("w1s1", [64, 9, 96], SDT, kind="ExternalInput")
    d_w1p0 = nc.dram_tensor("w1p0", [128, 3, 96], SDT, kind="ExternalInput")
    d_w1p1 = nc.dram_tensor("w1p1", [128, 3, 96], SDT, kind="ExternalInput")
    d_w2r0 = nc.dram_tensor("w2r0", [64, 2, 64], SDT, kind="ExternalInput")
    d_w2r1 = nc.dram_tensor("w2r1", [64, 2, 64], SDT, kind="ExternalInput")
    d_oh8r = nc.dram_tensor("oh8r", [64, 256], SDT, kind="ExternalInput")
    d_pat = nc.dram_tensor("patterns", [128, 4, 32], F32, kind="ExternalInput")
    d_patTr = nc.dram_tensor("patTr", [32, 512], SDT, kind="ExternalInput")
    d_ohrep = nc.dram_tensor("ohrep", [8, 8, 64], F32, kind="ExternalInput")
    d_zer = nc.dram_tensor("zer", [64, 3200], F32, kind="ExternalInput")
    d_out = nc.dram_tensor("out", [32, 64], F32, kind="ExternalOutput")

    with tile.TileContext(nc) as tc:
        with (
            tc.tile_pool(name="big", bufs=1) as big,
            tc.tile_pool(name="tmp", bufs=4) as tmp,
            tc.tile_pool(name="psum", bufs=3, space="PSUM") as ps,
            tc.tile_pool(name="psumj", bufs=4, space="PSUM") as psj,
            tc.tile_pool(name="psume", bufs=1, space="PSUM") as pse,
        ):
            _ps_n = [0]

            def pst(shape):
                _ps_n[0] += 1
                return ps.tile(shape, F32, tag="ps", name=f"ps{_ps_n[0]}")

            # ---- persistent SBUF ----
            T32 = big.tile([64, 64, 10, 10], F32, tag="T32")
            MT32 = big.tile([128, 64, 10, 10], SDT, tag="MT32")
            MH32 = big.tile([64, 4, 8, 64], SDT, tag="MH32")  # [2par*h, j, kk8, pix]
            T16 = big.tile([64, 64, 10, 10], F32, tag="T16")
            MT16 = big.tile([128, 64, 10, 10], WDT, tag="MT16")
            MH16 = big.tile([64, 4, 8, 64], WDT, tag="MH16")

            w1T = big.tile([64, 9, 64], F32, tag="w1T")
            pk64 = big.tile([64, 161], F32, tag="pk64")
            pk32 = big.tile([32, 577], F32, tag="pk32")
            b1 = pk64[:, 0:1]
            c2wT = pk64[:, 1:33]
            ident = pk64[:, 33:97]
            b2 = pk32[:, 0:1]
            c2w_oc = pk32[:, 1:65]
            patT = pk32[:, 65:577]
            r0w1T = big.tile([64, 9, 32], F32, tag="r0w1T")
            r1w1T = big.tile([64, 9, 32], F32, tag="r1w1T")
            r0w2T = big.tile([64, 64], F32, tag="r0w2T")   # parity-dup rows
            r1w2T = big.tile([64, 64], F32, tag="r1w2T")
            # conv weights in par-padded layouts: singles [64, 9, 96]
            # (cols w|0|w, par slice at 32*par), packed [128, 3, 96],
            # w2 [64(2par*h), 2(par), 64] -- everything partition-base 0
            w1s0 = big.tile([64, 9, 96], SDT, tag="w1s0")
            w1s1 = big.tile([64, 9, 96], SDT, tag="w1s1")
            w1p0 = big.tile([128, 3, 96], SDT, tag="w1p0")
            w1p1 = big.tile([128, 3, 96], SDT, tag="w1p1")
            w2r0 = big.tile([64, 2, 64], SDT, tag="w2r0")
            w2r1 = big.tile([64, 2, 64], SDT, tag="w2r1")
            oh8r = big.tile([64, 256], SDT, tag="oh8r")
            w1s0b = big.tile([64, 9, 96], WDT, tag="w1s0b")
            w1s1b = big.tile([64, 9, 96], WDT, tag="w1s1b")
            w1p0b = big.tile([128, 3, 96], WDT, tag="w1p0b")
            w1p1b = big.tile([128, 3, 96], WDT, tag="w1p1b")
            w2r0b = big.tile([64, 2, 64], WDT, tag="w2r0b")
            w2r1b = big.tile([64, 2, 64], WDT, tag="w2r1b")
            pat = big.tile([128, 4, 32], F32, tag="pat")
            ohrep = big.tile([8, 8, 64], F32, tag="ohrep")
            ohrep_b = big.tile([8, 8, 64], BF16, tag="ohrep_b")
            ones64 = big.tile([64, 64], F32, tag="ones64")
            patTr = big.tile([32, 512], SDT, tag="patTr")
            ymr = big.tile([32, 64], SDT, tag="ymr")
            ohf8 = big.tile([8, 8, 64], BF16, tag="ohf8")
            mn8 = big.tile([8, 8, 1], F32, tag="mn8")

            x_pad = big.tile([64, 10, 10], F32, tag="x_pad")
            a_pad = big.tile([64, 10, 10], F32, tag="a_pad")
            m1a = big.tile([64, 64], F32, tag="m1a")
            m2a = big.tile([64, 64], F32, tag="m2a")
            m3 = big.tile([64, 64], F32, tag="m3")
            m1b = big.tile([64, 64], F32, tag="m1b")   # parity-dup at +32
            m2b = big.tile([64, 64], F32, tag="m2b")
            y1 = big.tile([64, 64], F32, tag="y1")
            y2 = big.tile([64, 64], F32, tag="y2")
            y3 = big.tile([64, 64], F32, tag="y3")
            y4 = big.tile([64, 64], F32, tag="y4")
            yout = big.tile([32, 64], F32, tag="yout")
            r_sb = big.tile([32, 64], F32, tag="r_sb")
            V = big.tile([64, 64], F32, tag="V")       # (c2w^T r) * m3
            P1 = big.tile([64, 512], F32, tag="P1")
            P2 = big.tile([64, 512], F32, tag="P2")
            ym = big.tile([32, 64, 1], F32, tag="ym")
            Gq = big.tile([64, 64, 8], F32, tag="Gq")
            G = big.tile([64, 64, 1], F32, tag="G")
            out_sb = big.tile([32, 64], F32, tag="out_sb")
            prodE = big.tile([64, 64, 64], SDT, tag="prodE")
            prodW = big.tile([64, 64, 64], F32, tag="prodW")

            # ---- loads: early-needed first per queue; Pool does no DMA ----
            sdma = nc.sync.dma_start
            adma = nc.scalar.dma_start
            sdma(out=x_pad[:, 1:9, 1:9],
                 in_=d_x[:].rearrange("c (y x) -> c y x", y=8))
            sdma(out=w1T[:], in_=d_w1T[:])
            sdma(out=pk64[:], in_=d_pk64[:])
            sdma(out=r0w1T[:], in_=d_r0w1T[:])
            sdma(out=w1s0[:], in_=d_w1s0[:])
            sdma(out=w1p0[:], in_=d_w1p0[:])
            sdma(out=r0w2T[:], in_=d_r0w2T[:])
            sdma(out=w2r0[:], in_=d_w2r0[:])
            sdma(out=pk32[:], in_=d_pk32[:])
            sdma(out=r1w1T[:], in_=d_r1w1T[:])
            sdma(out=w1s1[:], in_=d_w1s1[:])
            sdma(out=w1p1[:], in_=d_w1p1[:])
            sdma(out=r1w2T[:], in_=d_r1w2T[:])
            sdma(out=w2r1[:], in_=d_w2r1[:])
            sdma(out=oh8r[:], in_=d_oh8r[:])
            sdma(out=pat[:], in_=d_pat[:])
            sdma(out=ohrep[:], in_=d_ohrep[:])
            sdma(out=patTr[:], in_=d_patTr[:])

            # ---- memsets: T frames zeroed early on Pool+DVE; Act's queue
            # stays clear for the forward-pass relus ----
            nc.vector.memset(x_pad[:, :, 0:1], 0.0)
            nc.vector.memset(x_pad[:, :, 9:10], 0.0)
            nc.vector.memset(x_pad[:, 0, 1:9], 0.0)
            nc.vector.memset(x_pad[:, 9, 1:9], 0.0)
            nc.vector.memset(a_pad[:], 0.0)
            nc.vector.memset(ones64[:], 1.0)
            nc.gpsimd.memset(T32[:, 0:32, :, :], 0.0)
            nc.scalar.memzero(T32[:, 32:64, :, :])
            nc.gpsimd.memset(T16[:, 0:32, :, :], 0.0)
            nc.vector.memset(T16[:, 32:64, :, :], 0.0)
            # MT lower borders (upper halves are rewritten by the shift-copy)
            nc.scalar.memzero(MT16[0:64, :, 0, :])
            nc.scalar.memzero(MT16[0:64, :, 9, :])
            nc.gpsimd.memset(MT16[0:64, :, 1:9, 0], 0.0)
            nc.gpsimd.memset(MT16[0:64, :, 1:9, 9], 0.0)
            for reg in ((slice(None), 0, slice(None)),
                        (slice(None), 9, slice(None)),
                        (slice(None), slice(1, 9), 0),
                        (slice(None), slice(1, 9), 9)):
                nc.vector.tensor_tensor(
                    out=MT32[(slice(0, 64),) + reg],
                    in0=T32[(slice(None),) + reg],
                    in1=T32[(slice(None),) + reg], op=ALU.mult)
            nc.vector.tensor_copy(w1s0b[:], w1s0[:])
            nc.vector.tensor_copy(w1p0b[:], w1p0[:])
            nc.vector.tensor_copy(w2r0b[:], w2r0[:])
            nc.gpsimd.tensor_copy(w1s1b[:], w1s1[:])
            nc.gpsimd.tensor_copy(w1p1b[:], w1p1[:])
            nc.gpsimd.tensor_copy(w2r1b[:], w2r1[:])

            TAPS = [(ky, kx) for ky in range(3) for kx in range(3)]

            def conv9(out_ps, wT_d, src_pad, M):
                for t, (ky, kx) in enumerate(TAPS):
                    nc.tensor.matmul(
                        out_ps, wT_d[:, t, :M],
                        src_pad[:, ky:ky + 8, kx:kx + 8],
                        start=(t == 0), stop=(t == 8))

            # ================= tangent init =================
            # T[p, kk=(iy,ix), iy+ky, ix+kx] = VW[p, (2-ky,2-kx), kk],
            # scattered straight from PSUM. Also warms up the PE pstate
            # before the forward pass's serial chain.
            for ky in range(3):
                vw3v = pst([64, 3, 64])
                vw3q = pst([64, 3, 64])
                for kx in range(3):
                    t_src = (2 - ky) * 3 + (2 - kx)
                    nc.tensor.matmul(vw3v[:, kx, :], w1T[:, t_src, :],
                                     ones64[:], start=True, stop=True)
                    nc.tensor.matmul(vw3q[:, kx, :], w1T[:, t_src, :],
                                     x_pad[:, 1:9, 1:9],
                                     start=True, stop=True)
                nc.vector.tensor_copy(
                    _raw_ap(T32[:], ky * 10, [[1, 3], [810, 8], [101, 8]]),
                    _raw_ap(vw3v[:], 0, [[64, 3], [8, 8], [1, 8]]))
                nc.vector.tensor_copy(
                    _raw_ap(T16[:], ky * 10, [[1, 3], [810, 8], [101, 8]]),
                    _raw_ap(vw3q[:], 0, [[64, 3], [8, 8], [1, 8]]))

            # ================= forward head =================
            y1p = pst([64, 64])
            conv9(y1p[:], w1T, x_pad, 64)
            nc.vector.tensor_scalar(out=y1[:], in0=y1p[:], scalar1=b1,
                                    scalar2=None, op0=ALU.add)
            nc.vector.tensor_scalar(out=m1a[:], in0=y1[:], scalar1=0.0,
                                    scalar2=None, op0=ALU.is_gt)
            nc.scalar.activation(
                out=a_pad[:, 1:9, 1:9],
                in_=y1[:].rearrange("c (y x) -> c y x", y=8), func=ACTF.Relu)

            def fwd_block(w1T_d, w2T_d, mb, ma_next, y_in, y_out):
                hp = pst([32, 64])
                conv9(hp[:], w1T_d, a_pad, 32)
                nc.vector.tensor_scalar(out=mb[0:32, :], in0=hp[:], scalar1=0.0,
                                        scalar2=None, op0=ALU.is_gt)
                sdma(out=mb[32:64, :], in_=mb[0:32, :])
                bh = tmp.tile([32, 64], F32, tag="bh")
                nc.vector.tensor_scalar_max(bh[:], hp[:], 0.0)
                up = pst([64, 64])
                nc.tensor.matmul(up[:], w2T_d[0:32, 0:64], bh[:],
                                 start=True, stop=True)
                nc.vector.tensor_tensor(out=y_out[:], in0=y_in[:], in1=up[:],
                                        op=ALU.add)
                nc.vector.tensor_scalar(out=ma_next[:], in0=y_out[:],
                                        scalar1=0.0, scalar2=None, op0=ALU.is_gt)

            # ================= tangent stage phases =================
            # cfg = (Tt, MTt, MHt, w1s, w1p, w2T, cast, dma_q, acc_eng, minr)
            def phase_mask_j(s, cfgs, ma, j):
                    for cf in cfgs:
                        Tt, MTt, dq, meng = cf[0], cf[1], cf[7], cf[11]
                        o_lo, o_hi = _win(2 * j - s - 1, 2 * j + s + 3, cf[9])
                        m_lo, m_hi = max(0, o_lo - 1), min(8, o_hi + 1)
                        meng.tensor_tensor(
                            out=MTt[0:64, 16 * j:16 * j + 16,
                                    1 + m_lo:1 + m_hi, 1:9],
                            in0=Tt[:, 16 * j:16 * j + 16,
                                   1 + m_lo:1 + m_hi, 1:9],
                            in1=ma[:, 8 * m_lo:8 * m_hi].rearrange(
                                "p (k y x) -> p k y x", k=1, y=m_hi - m_lo)
                                .broadcast_to((64, 16, m_hi - m_lo, 8)),
                            op=ALU.mult)
                        # upper half = +1-flat-shift of the lower via DMA
                        dq(out=_raw_ap(MTt[64:128, :, :, :], 1600 * j,
                                       [[1, 1599]]),
                           in_=_raw_ap(MTt[0:64, :, :, :], 1600 * j + 1,
                                       [[1, 1599]]))

            def phase_mask(s, cfgs, ma):
                for j in range(4):
                    phase_mask_j(s, cfgs, ma, j)

            def phase_conva(s, cfgs):
                for j in range(4):
                    for cf in cfgs:
                        MTt, w1s_t, w1p_t, cast, minr = (
                            cf[1], cf[3], cf[4], cf[6], cf[9])
                        o_lo, o_hi = _win(2 * j - s - 1, 2 * j + s + 3, minr)
                        rows = o_hi - o_lo
                        _ps_n[0] += 1
                        pj = psj.tile([64, 8, rows, 8], F32, tag="pj",
                                      name=f"pj{_ps_n[0]}")
                        for par in range(2):
                            qq = 2 * j + par
                            # 3 single (taps (ky,2), K=64) + 3 packed
                            # (taps (ky,0)+(ky,1), K=128) streams; the
                            # par-padded lhsT slice routes par outputs to
                            # partition halves of one accumulation region
                            for ky in range(3):
                                nc.tensor.matmul(
                                    pj[:],
                                    cast(w1s_t[:, 3 * ky + 2,
                                               32 * par:32 * par + 64]),
                                    cast(MTt[0:64, 8 * qq:8 * qq + 8,
                                             ky + o_lo:ky + o_hi, 2:10]),
                                    start=(par == 0 and ky == 0), stop=False)
                                nc.tensor.matmul(
                                    pj[:],
                                    cast(w1p_t[:, ky,
                                               32 * par:32 * par + 64]),
                                    cast(MTt[0:128, 8 * qq:8 * qq + 8,
                                             ky + o_lo:ky + o_hi, 0:8]),
                                    start=False, stop=(par == 1 and ky == 2))
                        cf[10].append(pj)

            def phase_mh_convb_acc(s, cfgs, mb, after_j=None):
                for j in range(4):
                    for cf in cfgs:
                        MHt, minr = cf[2], cf[9]
                        o_lo, o_hi = _win(2 * j - s - 1, 2 * j + s + 3, minr)
                        rows = o_hi - o_lo
                        pj = cf[10][j]
                        nc.vector.tensor_tensor(
                            out=MHt[:, j, :, 8 * o_lo:8 * o_hi],
                            in0=pj[:].rearrange("p k r x -> p k (r x)"),
                            in1=mb[:, 8 * o_lo:8 * o_hi].rearrange(
                                "p (k m) -> p k m", k=1)
                                .broadcast_to((64, 8, 8 * rows)),
                            op=ALU.mult)
                for qq in range(8):
                    if after_j is not None and qq >= 2 and qq % 2 == 0:
                        after_j(qq // 2 - 1)
                    j, par = qq // 2, qq % 2
                    for cf in cfgs:
                        (Tt, MTt, MHt, w1s_t, w1p_t, w2T_t,
                         cast, dq, aeng, minr, _pjs, _meng) = cf
                        q_lo, q_hi = _win(qq - s - 1, qq + s + 2, minr)
                        rows = q_hi - q_lo
                        uq = pst([64, 8, rows, 8])
                        nc.tensor.matmul(
                            uq[:].rearrange("p k r x -> p k (r x)"),
                            cast(w2T_t[:, par, :]),
                            cast(MHt[:, j, :, 8 * q_lo:8 * q_hi]),
                            start=True, stop=True)
                        if aeng is nc.gpsimd:
                            # GpSimd can't read PSUM: stage uq through SBUF
                            # on Act, then add on Pool (keeps DVE free)
                            uqsb = tmp.tile([64, 8, rows, 8], F32, tag="uqsb")
                            nc.scalar.copy(uqsb[:], uq[:])
                            acc_in = uqsb
                        else:
                            acc_in = uq
                        aeng.tensor_tensor(
                            out=Tt[:, 8 * qq:8 * qq + 8,
                                   1 + q_lo:1 + q_hi, 1:9],
                            in0=Tt[:, 8 * qq:8 * qq + 8,
                                   1 + q_lo:1 + q_hi, 1:9],
                            in1=acc_in[:],
                            op=ALU.add)
                if after_j is not None:
                    after_j(3)

            def w_cast(ap):
                return ap

            cfgs1 = [
                [T32, MT32, MH32, w1s0, w1p0, w2r0, w_cast,
                 sdma, nc.vector, s_min_rows, [], nc.vector],
                [T16, MT16, MH16, w1s0b, w1p0b, w2r0b, w_cast,
                 adma, nc.vector, 0, [], nc.gpsimd],
            ]
            cfgs2 = [
                [T32, MT32, MH32, w1s1, w1p1, w2r1, w_cast,
                 sdma, nc.vector, s_min_rows, [], nc.vector],
                [T16, MT16, MH16, w1s1b, w1p1b, w2r1b, w_cast,
                 adma, nc.vector, 0, [], nc.gpsimd],
            ]

            # ---- interleaved emission: fwd blocks fill PE gaps ----
            fwd_block(r0w1T, r0w2T, m1b, m2a, y1, y2)
            nc.scalar.activation(
                out=a_pad[:, 1:9, 1:9],
                in_=y2[:].rearrange("c (y x) -> c y x", y=8), func=ACTF.Relu)
            phase_mask(1, cfgs1, m1a)
            phase_conva(1, cfgs1)
            fwd_block(r1w1T, r1w2T, m2b, m3, y2, y3)
            nc.scalar.activation(out=y4[:], in_=y3[:], func=ACTF.Relu)
            yop = pst([32, 64])
            nc.tensor.matmul(yop[:], c2wT, y4[:], start=True, stop=True)
            nc.vector.tensor_scalar(out=yout[:], in0=yop[:], scalar1=b2,
                                    scalar2=None, op0=ALU.add)
            phase_mh_convb_acc(
                1, cfgs1, m1b,
                after_j=lambda j: phase_mask_j(2, cfgs2, m2a, j))

            # ================= hopfield helper =================
            def hopfield(y_ap, P, fast):
                lg = pst([64, 512])
                if fast:
                    nc.tensor.matmul(lg[:], y_ap, patTr[:],
                                     start=True, stop=True)
                else:
                    nc.tensor.matmul(lg[:], y_ap, patT, start=True, stop=True)
                ssum = tmp.tile([64, 1], F32, tag="ssum")
                # logits are tame (|lg|/sqrt(32) < 40): skip max-stabilization
                nc.scalar.activation(out=P[:], in_=lg[:], func=ACTF.Exp,
                                     bias=0.0, scale=ISQRT32, accum_out=ssum[:])
                rs = tmp.tile([64, 1], F32, tag="rs")
                nc.vector.reciprocal(rs[:], ssum[:])
                nc.scalar.mul(P[:], P[:], rs[:])
                yq = pse.tile([32, 64], F32, tag="et8", name=f"yq{_ps_n[0]}")
                for qc in range(4):
                    ptp = pst([128, 64])
                    pt = tmp.tile([128, 64], F32, tag="pt")
                    nc.tensor.transpose(ptp[:], P[:, 128 * qc:128 * (qc + 1)],
                                        ident)
                    if qc % 2 == 0:
                        nc.vector.tensor_copy(pt[:], ptp[:])
                    else:
                        nc.scalar.copy(pt[:], ptp[:])
                    nc.tensor.matmul(yq[:], pat[:, qc, :], pt[:],
                                     start=(qc == 0), stop=(qc == 3))
                return yq

            yq1 = hopfield(yout[:], P1, False)
            nc.vector.tensor_tensor(out=r_sb[:], in0=yout[:], in1=yq1[:],
                                    op=ALU.subtract)
            rps = pst([64, 64])
            nc.tensor.matmul(rps[:], c2w_oc, r_sb[:], start=True, stop=True)
            nc.vector.tensor_tensor(out=V[:], in0=rps[:], in1=m3[:],
                                    op=ALU.mult)
            phase_conva(2, cfgs2)

            # ================= C2 + routing + scatter =================
            # e_total[i,m] = sum_c V[c,m] * T32[c,i,m]; the prodE/et8
            # chunks are emitted inside stage-2's acc phase as each T32
            # chunk finalizes
            et8 = pse.tile([32, 8, 64], F32, tag="et8", name="et8")

            def emit_prode(j):
                for qq in (2 * j, 2 * j + 1):
                    peng = nc.vector if qq % 2 == 0 else nc.gpsimd
                    peng.tensor_tensor(
                        out=prodE[:, 8 * qq:8 * qq + 8, :]
                            .rearrange("p k (y x) -> p k y x", y=8),
                        in0=T32[:, 8 * qq:8 * qq + 8, 1:9, 1:9],
                        in1=V[:].rearrange("p (k y x) -> p k y x", k=1, y=8)
                            .broadcast_to((64, 8, 8, 8)),
                        op=ALU.mult)
                    # partition-sum lands on psum partition qq via one-hot
                    nc.tensor.matmul(
                        et8[:].rearrange("p k m -> p (k m)"),
                        oh8r[:, 32 * qq:32 * qq + 32],
                        prodE[:, 8 * qq:8 * qq + 8, :]
                            .rearrange("p k m -> p (k m)"),
                        start=(qq == 0), stop=(qq == 7))

            phase_mh_convb_acc(2, cfgs2, m2b, after_j=emit_prode)
            nc.vector.tensor_reduce(out=mn8[:], in_=et8[0:8, :, :], axis=AX.X,
                                    op=ALU.min)
            nc.vector.tensor_tensor(out=ohf8[:], in0=et8[0:8, :, :],
                                    in1=mn8[:].broadcast_to((8, 8, 64)),
                                    op=ALU.is_equal)
            nc.gpsimd.tensor_copy(ohrep_b[:], ohrep[:])
            # scatter: prodW[c, m, i] = T16[c, i, m] * onehot[i, m], then
            # per-chunk partial i-sums Gq -> G
            repsb = big.tile([64, 8, 8, 64], F32, tag="repsb")
            for qq in range(8):
                rep = pst([64, 8, 64])
                nc.tensor.matmul(
                    rep[:], ohrep_b[:, qq, :],
                    ohf8[:].rearrange("p k m -> p (k m)"),
                    start=True, stop=True)
                nc.scalar.copy(repsb[:, qq, :, :], rep[:])
                dst = _raw_ap(prodW[:], 8 * qq, [[1, 8], [512, 8], [64, 8]])
                weng = nc.vector if qq % 2 == 0 else nc.gpsimd
                weng.tensor_tensor(
                    out=dst,
                    in0=T16[:, 8 * qq:8 * qq + 8, 1:9, 1:9],
                    in1=repsb[:, qq, :, :].rearrange(
                        "p k (y x) -> p k y x", y=8),
                    op=ALU.mult)
                if qq % 2 == 1:
                    nc.vector.tensor_reduce(
                        out=Gq[:, :, qq // 2:qq // 2 + 1],
                        in_=prodW[:, :, 8 * qq - 8:8 * qq + 8],
                        axis=AX.X, op=ALU.add)
            nc.vector.tensor_reduce(out=G[:], in_=Gq[:, :, 0:4],
                                    axis=AX.X, op=ALU.add)
            nc.vector.tensor_tensor(
                out=G[:, :, 0], in0=G[:, :, 0], in1=m3[:], op=ALU.mult)
            ymp = pst([32, 64])
            for m0 in (0, 32):
                nc.tensor.matmul(ymp[:, m0:m0 + 32], c2wT,
                                 G[:, m0:m0 + 32, 0], start=True, stop=True)
            nc.vector.tensor_copy(ymr[:], ymp[:])

            yq2 = hopfield(ymr[:], P2, True)
            nc.vector.tensor_copy(out_sb[:], yq2[:])
            sdma(out=d_out[:], in_=out_sb[:])

    nc.compile()
    return nc


def _prep_weights(inputs):
    f = np.float32
    w1 = np.asarray(inputs['conv1_w'], f)
    w1t = w1.transpose(2, 3, 1, 0).reshape(9, 64, 64)         # [tap, c, o]
    r0 = np.asarray(inputs['res0_w1'], f).transpose(2, 3, 1, 0).reshape(9, 64, 32)
    r1 = np.asarray(inputs['res1_w1'], f).transpose(2, 3, 1, 0).reshape(9, 64, 32)
    r0w2 = np.asarray(inputs['res0_w2'], f)[:, :, 0, 0].T      # [32, 64]
    r1w2 = np.asarray(inputs['res1_w2'], f)[:, :, 0, 0].T
    pats = np.asarray(inputs['patterns'], f)

    def pack_p(r):   # [128, 3, 32]: parts 0-63 taps (ky,0), 64-127 taps (ky,1)
        return np.concatenate([r[[0, 3, 6]].transpose(1, 0, 2),
                               r[[1, 4, 7]].transpose(1, 0, 2)], axis=0)

    def dup2(w2):    # [64, 64]: parity-dup rows
        return np.concatenate([w2, w2], axis=0)

    def pad96(w):    # [P, T, 32] -> [P, T, 96] cols (w | 0 | w)
        P, T, _ = w.shape
        z = np.zeros((P, T, 96), f)
        z[:, :, 0:32] = w
        z[:, :, 64:96] = w
        return np.ascontiguousarray(z)

    def w2pad(w2):   # [32, 64] -> [64(2par*h), 2(par), 64]: par-selecting
        z = np.zeros((64, 2, 64), f)
        z[0:32, 0, :] = w2
        z[32:64, 1, :] = w2
        return np.ascontiguousarray(z)

    c = np.ascontiguousarray
    # pk64 = [b1 | c2w^T | I64 | oh8], pk32 = [b2 | c2w]
    pk64 = np.concatenate([
        np.asarray(inputs['conv1_b'], f).reshape(64, 1),
        np.asarray(inputs['conv2_w'], f)[:, :, 0, 0].T,
        np.eye(64, dtype=f),
        np.broadcast_to(np.eye(8, dtype=f), (64, 8, 8)).reshape(64, 64),
    ], axis=1)
    pk32 = np.concatenate([
        np.asarray(inputs['conv2_b'], f).reshape(32, 1),
        np.asarray(inputs['conv2_w'], f)[:, :, 0, 0],
        pats.T,
    ], axis=1)
    base = {
        'w1T': c(w1t.transpose(1, 0, 2)),
        'pk64': c(pk64),
        'pk32': c(pk32),
        'r0w1T': c(r0.transpose(1, 0, 2)),
        'w1s0': pad96(r0.transpose(1, 0, 2)),
        'w1s1': pad96(r1.transpose(1, 0, 2)),
        'w1p0': pad96(pack_p(r0)),
        'w1p1': pad96(pack_p(r1)),
        'w2r0': w2pad(r0w2),
        'w2r1': w2pad(r1w2),
        'oh8r': c(np.broadcast_to(np.concatenate(
            [np.eye(8, dtype=f), np.zeros((8, 24), f)], axis=1),
            (64, 8, 32)).reshape(64, 256)),
        'r0w2T': c(dup2(r0w2)),
        'r1w1T': c(r1.transpose(1, 0, 2)),
        'r1w2T': c(dup2(r1w2)),
        'patterns': c(pats.reshape(4, 128, 32).transpose(1, 0, 2)),
        'patternsT': c(pats.T),
        'patTr': c(pats.T),
        'zer': np.zeros((64, 3200), f),
        'ohrep': c(np.broadcast_to(np.eye(8, dtype=f)[:, :, None], (8, 8, 64))),
    }
    return base


def make_in_maps(inputs):
    x = np.asarray(inputs['x'], np.float32)
    base = _prep_weights(inputs)
    return [dict(base, x=np.ascontiguousarray(x[b].reshape(64, 64)))
            for b in range(8)]


def kernel(**inputs):
    _lazy_imports()
    from concourse.bass_utils import run_bass_kernel_spmd
    if 'nc' not in _CACHE:
        _CACHE['nc'] = build_nc()
    nc = _CACHE['nc']
    in_maps = make_in_maps(inputs)
    res = run_bass_kernel_spmd(nc, in_maps, list(range(8)))
    _CACHE['last_result'] = res
    out = np.stack([res.results[b]['out'].reshape(32, 8, 8) for b in range(8)])
    return out.astype(np.float32)


# revision 55
# speedup vs baseline: 1.0865x; 1.0301x over previous
"""Trainium2 Bass kernel for nn_Block2_87144886436578.

Reformulation: the reference materializes per-sample jacobians
J[o,m,c,i] = d propagate(x)[o,m] / d x[c,i] but only ever uses two
contractions of J:
  S[o,m,i]  = sum_c J[o,m,c,i]          (-> e_total -> argmin routing)
  Wt[o,m,i] = sum_c x[c,i] J[o,m,c,i]   (-> routed scatter y_masked)
Both are forward-mode JVPs whose input tangents live on a single pixel i:
  v_i = ones over channels at pixel i,  w_i = x[:, i] at pixel i.
So per sample we propagate 2x64 tangents through the ReLU-linearized conv
stack (masks from one forward pass). Batch is data-parallel: sample b ->
core b (8 cores).

Dtypes: S (v-tangent) half runs its convs in f32r (1 cycle/row on PE for
N>=256 vs 4 for fp32); the Wt half's conv inputs are bf16.

Support restriction: tangent i's support after stage s is a (2s+3)-row
window around its row iy, so every mask / conv / accumulate only touches
that window (windows only grow stage to stage, so stale rows outside a
window are always zero).

Emission order interleaves the forward pass with the tangent stages so
the in-order PE queue never stalls on the forward pass's serial
dependency chain.

Layout per half: tangents T [64 part(ch), 64 kk, 10, 10] zero-padded
frames; masked tangents MT [128, ...] where partitions 64-127 hold a
+1-column pre-shift of 0-63 (produced by a flat DMA copy at offset +1),
enabling tap-pair K=128 packing of the 3x3 convs: 6 PE streams instead
of 9. e_total is accumulated into an [8, 8, 64] PSUM tile via one-hot
column lhsT so the argmin reduction runs on 8 partitions.
"""
import os
import numpy as np

F32 = None  # set in _lazy_imports
_CACHE = {}

# S-half conv dtype: "f32r" (4x faster on PE) or "f32" (exact fallback).
S_MODE = os.environ.get('BASS_S_MODE', 'f32r')
# Wt-half conv-input dtype: "bf16" (4x faster) or "f32".
W_MODE = os.environ.get('BASS_W_MODE', 'bf16')


def _lazy_imports():
    global bacc, bass, tile, mybir, F32, BF16, F32R, AX, ALU, ACTF
    import concourse.bacc as bacc
    import concourse.bass as bass
    import concourse.tile as tile
    import concourse.mybir as mybir
    F32 = mybir.dt.float32
    BF16 = mybir.dt.bfloat16
    F32R = mybir.dt.float32r
    AX = mybir.AxisListType
    ALU = mybir.AluOpType
    ACTF = mybir.ActivationFunctionType


ISQRT32 = 0.17677669529663687  # 1/sqrt(32)


def _raw_ap(t_ap, extra_offset, dims):
    """AP on t_ap's tensor: keep partition dim, replace free dims."""
    return bass.AP(tensor=t_ap.tensor, offset=t_ap.offset + extra_offset,
                   ap=[list(t_ap.ap[0])] + [list(d) for d in dims])


def _win(lo, hi, min_rows=0):
    lo, hi = max(0, lo), min(8, hi)
    while hi - lo < min_rows:
        if hi < 8:
            hi += 1
        else:
            lo -= 1
    return lo, hi


def build_nc():
    _lazy_imports()
    nc = bacc.Bacc("TRN2", target_bir_lowering=False, debug=True)

    # f32r-consumed tensors are declared float32r end-to-end (the BIR
    # verifier requires every producer of f32r-matmul operands to round
    # to f32r); the forward pass keeps separate fp32 weight copies.
    SDT = F32R if S_MODE == 'f32r' else F32
    s_min_rows = 4 if S_MODE == 'f32r' else 0
    WDT = {'bf16': BF16, 'f32': F32}[W_MODE]

    # ---- DRAM I/O (per-core; weights replicated across cores) ----
    d_x = nc.dram_tensor("x", [64, 64], F32, kind="ExternalInput")
    d_w1T = nc.dram_tensor("w1T", [64, 9, 64], F32, kind="ExternalInput")
    d_pk64 = nc.dram_tensor("pk64", [64, 161], F32, kind="ExternalInput")
    d_pk32 = nc.dram_tensor("pk32", [32, 577], F32, kind="ExternalInput")
    d_r0w1T = nc.dram_tensor("r0w1T", [64, 9, 32], F32, kind="ExternalInput")
    d_r0w2T = nc.dram_tensor("r0w2T", [64, 64], F32, kind="ExternalInput")
    d_r1w1T = nc.dram_tensor("r1w1T", [64, 9, 32], F32, kind="ExternalInput")
    d_r1w2T = nc.dram_tensor("r1w2T", [64, 64], F32, kind="ExternalInput")
    d_w1s0 = nc.dram_tensor("w1s0", [64, 9, 96], SDT, kind="ExternalInput")
    d_w1s1 = nc.dram_tensor("w1s1", [64, 9, 96], SDT, kind="ExternalInput")
    d_w1p0 = nc.dram_tensor("w1p0", [128, 3, 96], SDT, kind="ExternalInput")
    d_w1p1 = nc.dram_tensor("w1p1", [128, 3, 96], SDT, kind="ExternalInput")
    d_w2r0 = nc.dram_tensor("w2r0", [64, 2, 64], SDT, kind="ExternalInput")
    d_w2r1 = nc.dram_tensor("w2r1", [64, 2, 64], SDT, kind="ExternalInput")
    d_oh8r = nc.dram_tensor("oh8r", [64, 256], SDT, kind="ExternalInput")
    d_pat = nc.dram_tensor("patterns", [128, 4, 32], F32, kind="ExternalInput")
    d_patTr = nc.dram_tensor("patTr", [32, 512], SDT, kind="ExternalInput")
    d_ohrep = nc.dram_tensor("ohrep", [8, 8, 64], F32, kind="ExternalInput")
    d_zer = nc.dram_tensor("zer", [64, 3200], F32, kind="ExternalInput")
    d_zerr = nc.dram_tensor("zerr", [64, 6400], SDT, kind="ExternalInput")
    d_zerb = nc.dram_tensor("zerb", [64, 6400], WDT, kind="ExternalInput")
    d_out = nc.dram_tensor("out", [32, 64], F32, kind="ExternalOutput")

    with tile.TileContext(nc) as tc:
        with (
            tc.tile_pool(name="big", bufs=1) as big,
            tc.tile_pool(name="tmp", bufs=4) as tmp,
            tc.tile_pool(name="psum", bufs=3, space="PSUM") as ps,
            tc.tile_pool(name="psumj", bufs=4, space="PSUM") as psj,
            tc.tile_pool(name="psume", bufs=1, space="PSUM") as pse,
        ):
            _ps_n = [0]

            def pst(shape):
                _ps_n[0] += 1
                return ps.tile(shape, F32, tag="ps", name=f"ps{_ps_n[0]}")

            # ---- persistent SBUF ----
            T32 = big.tile([64, 64, 10, 10], F32, tag="T32")
            MT32 = big.tile([128, 64, 10, 10], SDT, tag="MT32")
            MH32 = big.tile([64, 4, 8, 64], SDT, tag="MH32")  # [2par*h, j, kk8, pix]
            T16 = big.tile([64, 64, 10, 10], F32, tag="T16")
            MT16 = big.tile([128, 64, 10, 10], WDT, tag="MT16")
            MH16 = big.tile([64, 4, 8, 64], WDT, tag="MH16")

            w1T = big.tile([64, 9, 64], F32, tag="w1T")
            pk64 = big.tile([64, 161], F32, tag="pk64")
            pk32 = big.tile([32, 577], F32, tag="pk32")
            b1 = pk64[:, 0:1]
            c2wT = pk64[:, 1:33]
            ident = pk64[:, 33:97]
            b2 = pk32[:, 0:1]
            c2w_oc = pk32[:, 1:65]
            patT = pk32[:, 65:577]
            r0w1T = big.tile([64, 9, 32], F32, tag="r0w1T")
            r1w1T = big.tile([64, 9, 32], F32, tag="r1w1T")
            r0w2T = big.tile([64, 64], F32, tag="r0w2T")   # parity-dup rows
            r1w2T = big.tile([64, 64], F32, tag="r1w2T")
            # conv weights in par-padded layouts: singles [64, 9, 96]
            # (cols w|0|w, par slice at 32*par), packed [128, 3, 96],
            # w2 [64(2par*h), 2(par), 64] -- everything partition-base 0
            w1s0 = big.tile([64, 9, 96], SDT, tag="w1s0")
            w1s1 = big.tile([64, 9, 96], SDT, tag="w1s1")
            w1p0 = big.tile([128, 3, 96], SDT, tag="w1p0")
            w1p1 = big.tile([128, 3, 96], SDT, tag="w1p1")
            w2r0 = big.tile([64, 2, 64], SDT, tag="w2r0")
            w2r1 = big.tile([64, 2, 64], SDT, tag="w2r1")
            oh8r = big.tile([64, 256], SDT, tag="oh8r")
            w1s0b = big.tile([64, 9, 96], WDT, tag="w1s0b")
            w1s1b = big.tile([64, 9, 96], WDT, tag="w1s1b")
            w1p0b = big.tile([128, 3, 96], WDT, tag="w1p0b")
            w1p1b = big.tile([128, 3, 96], WDT, tag="w1p1b")
            w2r0b = big.tile([64, 2, 64], WDT, tag="w2r0b")
            w2r1b = big.tile([64, 2, 64], WDT, tag="w2r1b")
            pat = big.tile([128, 4, 32], F32, tag="pat")
            ohrep = big.tile([8, 8, 64], F32, tag="ohrep")
            ohrep_b = big.tile([8, 8, 64], BF16, tag="ohrep_b")
            ones64 = big.tile([64, 64], F32, tag="ones64")
            patTr = big.tile([32, 512], SDT, tag="patTr")
            ymr = big.tile([32, 64], SDT, tag="ymr")
            ohf8 = big.tile([8, 8, 64], BF16, tag="ohf8")
            mn8 = big.tile([8, 8, 1], F32, tag="mn8")

            x_pad = big.tile([64, 10, 10], F32, tag="x_pad")
            a_pad = big.tile([64, 10, 10], F32, tag="a_pad")
            m1a = big.tile([64, 64], F32, tag="m1a")
            m2a = big.tile([64, 64], F32, tag="m2a")
            m3 = big.tile([64, 64], F32, tag="m3")
            m1b = big.tile([64, 64], F32, tag="m1b")   # parity-dup at +32
            m2b = big.tile([64, 64], F32, tag="m2b")
            y1 = big.tile([64, 64], F32, tag="y1")
            y2 = big.tile([64, 64], F32, tag="y2")
            y3 = big.tile([64, 64], F32, tag="y3")
            y4 = big.tile([64, 64], F32, tag="y4")
            yout = big.tile([32, 64], F32, tag="yout")
            r_sb = big.tile([32, 64], F32, tag="r_sb")
            V = big.tile([64, 64], F32, tag="V")       # (c2w^T r) * m3
            P1 = big.tile([64, 512], F32, tag="P1")
            P2 = big.tile([64, 512], F32, tag="P2")
            ym = big.tile([32, 64, 1], F32, tag="ym")
            Gq = big.tile([64, 64, 8], F32, tag="Gq")
            G = big.tile([64, 64, 1], F32, tag="G")
            out_sb = big.tile([32, 64], F32, tag="out_sb")
            prodE = big.tile([64, 64, 64], SDT, tag="prodE")
            prodW = big.tile([64, 64, 64], F32, tag="prodW")

            # ---- loads: early-needed first per queue; Pool does no DMA ----
            sdma = nc.sync.dma_start
            adma = nc.scalar.dma_start
            sdma(out=x_pad[:, 1:9, 1:9],
                 in_=d_x[:].rearrange("c (y x) -> c y x", y=8))
            sdma(out=w1T[:], in_=d_w1T[:])
            sdma(out=_raw_ap(MT32[0:64, :, :, :], 0, [[1, 6400]]),
                 in_=d_zerr[:])
            sdma(out=_raw_ap(MT16[0:64, :, :, :], 0, [[1, 6400]]),
                 in_=d_zerb[:])
            sdma(out=pk64[:], in_=d_pk64[:])
            sdma(out=r0w1T[:], in_=d_r0w1T[:])
            sdma(out=w1s0[:], in_=d_w1s0[:])
            sdma(out=w1p0[:], in_=d_w1p0[:])
            sdma(out=r0w2T[:], in_=d_r0w2T[:])
            sdma(out=w2r0[:], in_=d_w2r0[:])
            sdma(out=pk32[:], in_=d_pk32[:])
            sdma(out=r1w1T[:], in_=d_r1w1T[:])
            sdma(out=w1s1[:], in_=d_w1s1[:])
            sdma(out=w1p1[:], in_=d_w1p1[:])
            sdma(out=r1w2T[:], in_=d_r1w2T[:])
            sdma(out=w2r1[:], in_=d_w2r1[:])
            sdma(out=oh8r[:], in_=d_oh8r[:])
            sdma(out=pat[:], in_=d_pat[:])
            sdma(out=ohrep[:], in_=d_ohrep[:])
            sdma(out=patTr[:], in_=d_patTr[:])

            # ---- memsets: T frames zeroed early on Pool+DVE; Act's queue
            # stays clear for the forward-pass relus ----
            nc.vector.memset(x_pad[:, :, 0:1], 0.0)
            nc.vector.memset(x_pad[:, :, 9:10], 0.0)
            nc.vector.memset(x_pad[:, 0, 1:9], 0.0)
            nc.vector.memset(x_pad[:, 9, 1:9], 0.0)
            nc.vector.memset(a_pad[:], 0.0)
            nc.vector.memset(ones64[:], 1.0)
            nc.gpsimd.memset(T32[:, 0:32, :, :], 0.0)
            nc.scalar.memzero(T32[:, 32:64, :, :])
            nc.gpsimd.memset(T16[:, 0:32, :, :], 0.0)
            nc.vector.memset(T16[:, 32:64, :, :], 0.0)
            # MT lower borders (upper halves are rewritten by the shift-copy)
            nc.vector.tensor_copy(w1s0b[:], w1s0[:])
            nc.vector.tensor_copy(w1p0b[:], w1p0[:])
            nc.vector.tensor_copy(w2r0b[:], w2r0[:])
            nc.gpsimd.tensor_copy(w1s1b[:], w1s1[:])
            nc.gpsimd.tensor_copy(w1p1b[:], w1p1[:])
            nc.gpsimd.tensor_copy(w2r1b[:], w2r1[:])

            TAPS = [(ky, kx) for ky in range(3) for kx in range(3)]

            def conv9(out_ps, wT_d, src_pad, M):
                for t, (ky, kx) in enumerate(TAPS):
                    nc.tensor.matmul(
                        out_ps, wT_d[:, t, :M],
                        src_pad[:, ky:ky + 8, kx:kx + 8],
                        start=(t == 0), stop=(t == 8))

            # ================= tangent init =================
            # T[p, kk=(iy,ix), iy+ky, ix+kx] = VW[p, (2-ky,2-kx), kk],
            # scattered straight from PSUM. Also warms up the PE pstate
            # before the forward pass's serial chain.
            for ky in range(3):
                vw3v = pst([64, 3, 64])
                vw3q = pst([64, 3, 64])
                for kx in range(3):
                    t_src = (2 - ky) * 3 + (2 - kx)
                    nc.tensor.matmul(vw3v[:, kx, :], w1T[:, t_src, :],
                                     ones64[:], start=True, stop=True)
                    nc.tensor.matmul(vw3q[:, kx, :], w1T[:, t_src, :],
                                     x_pad[:, 1:9, 1:9],
                                     start=True, stop=True)
                nc.vector.tensor_copy(
                    _raw_ap(T32[:], ky * 10, [[1, 3], [810, 8], [101, 8]]),
                    _raw_ap(vw3v[:], 0, [[64, 3], [8, 8], [1, 8]]))
                nc.vector.tensor_copy(
                    _raw_ap(T16[:], ky * 10, [[1, 3], [810, 8], [101, 8]]),
                    _raw_ap(vw3q[:], 0, [[64, 3], [8, 8], [1, 8]]))

            # ================= forward head =================
            y1p = pst([64, 64])
            conv9(y1p[:], w1T, x_pad, 64)
            nc.vector.tensor_scalar(out=y1[:], in0=y1p[:], scalar1=b1,
                                    scalar2=None, op0=ALU.add)
            nc.vector.tensor_scalar(out=m1a[:], in0=y1[:], scalar1=0.0,
                                    scalar2=None, op0=ALU.is_gt)
            nc.scalar.activation(
                out=a_pad[:, 1:9, 1:9],
                in_=y1[:].rearrange("c (y x) -> c y x", y=8), func=ACTF.Relu)

            def fwd_block(w1T_d, w2T_d, mb, ma_next, y_in, y_out):
                hp = pst([32, 64])
                conv9(hp[:], w1T_d, a_pad, 32)
                nc.vector.tensor_scalar(out=mb[0:32, :], in0=hp[:], scalar1=0.0,
                                        scalar2=None, op0=ALU.is_gt)
                sdma(out=mb[32:64, :], in_=mb[0:32, :])
                bh = tmp.tile([32, 64], F32, tag="bh")
                nc.vector.tensor_scalar_max(bh[:], hp[:], 0.0)
                up = pst([64, 64])
                nc.tensor.matmul(up[:], w2T_d[0:32, 0:64], bh[:],
                                 start=True, stop=True)
                nc.vector.tensor_tensor(out=y_out[:], in0=y_in[:], in1=up[:],
                                        op=ALU.add)
                nc.vector.tensor_scalar(out=ma_next[:], in0=y_out[:],
                                        scalar1=0.0, scalar2=None, op0=ALU.is_gt)

            # ================= tangent stage phases =================
            # cfg = (Tt, MTt, MHt, w1s, w1p, w2T, cast, dma_q, acc_eng, minr)
            def phase_mask_j(s, cfgs, ma, j):
                    for cf in cfgs:
                        Tt, MTt, dq, meng = cf[0], cf[1], cf[7], cf[11]
                        m_lo, m_hi = _win(2 * j - s, 2 * j + s + 2)
                        meng.tensor_tensor(
                            out=MTt[0:64, 16 * j:16 * j + 16,
                                    1 + m_lo:1 + m_hi, 1:9],
                            in0=Tt[:, 16 * j:16 * j + 16,
                                   1 + m_lo:1 + m_hi, 1:9],
                            in1=ma[:, 8 * m_lo:8 * m_hi].rearrange(
                                "p (k y x) -> p k y x", k=1, y=m_hi - m_lo)
                                .broadcast_to((64, 16, m_hi - m_lo, 8)),
                            op=ALU.mult)
                        # upper half = +1-flat-shift of the lower via DMA
                        dq(out=_raw_ap(MTt[64:128, :, :, :], 1600 * j,
                                       [[1, 1599]]),
                           in_=_raw_ap(MTt[0:64, :, :, :], 1600 * j + 1,
                                       [[1, 1599]]))

            def phase_mask(s, cfgs, ma):
                for j in range(4):
                    phase_mask_j(s, cfgs, ma, j)

            def phase_conva(s, cfgs):
                for j in range(4):
                    for cf in cfgs:
                        MTt, w1s_t, w1p_t, cast, minr = (
                            cf[1], cf[3], cf[4], cf[6], cf[9])
                        o_lo, o_hi = _win(2 * j - s - 1, 2 * j + s + 3, minr)
                        rows = o_hi - o_lo
                        _ps_n[0] += 1
                        pj = psj.tile([64, 8, rows, 8], F32, tag="pj",
                                      name=f"pj{_ps_n[0]}")
                        for par in range(2):
                            qq = 2 * j + par
                            # 3 single (taps (ky,2), K=64) + 3 packed
                            # (taps (ky,0)+(ky,1), K=128) streams; the
                            # par-padded lhsT slice routes par outputs to
                            # partition halves of one accumulation region
                            for ky in range(3):
                                nc.tensor.matmul(
                                    pj[:],
                                    cast(w1s_t[:, 3 * ky + 2,
                                               32 * par:32 * par + 64]),
                                    cast(MTt[0:64, 8 * qq:8 * qq + 8,
                                             ky + o_lo:ky + o_hi, 2:10]),
                                    start=(par == 0 and ky == 0), stop=False)
                                nc.tensor.matmul(
                                    pj[:],
                                    cast(w1p_t[:, ky,
                                               32 * par:32 * par + 64]),
                                    cast(MTt[0:128, 8 * qq:8 * qq + 8,
                                             ky + o_lo:ky + o_hi, 0:8]),
                                    start=False, stop=(par == 1 and ky == 2))
                        cf[10].append(pj)

            def phase_mh_convb_acc(s, cfgs, mb, after_j=None):
                for j in range(4):
                    for cf in cfgs:
                        MHt, minr = cf[2], cf[9]
                        o_lo, o_hi = _win(2 * j - s - 1, 2 * j + s + 3, minr)
                        rows = o_hi - o_lo
                        pj = cf[10][j]
                        nc.vector.tensor_tensor(
                            out=MHt[:, j, :, 8 * o_lo:8 * o_hi],
                            in0=pj[:].rearrange("p k r x -> p k (r x)"),
                            in1=mb[:, 8 * o_lo:8 * o_hi].rearrange(
                                "p (k m) -> p k m", k=1)
                                .broadcast_to((64, 8, 8 * rows)),
                            op=ALU.mult)
                for qq in range(8):
                    if after_j is not None and qq >= 2 and qq % 2 == 0:
                        after_j(qq // 2 - 1)
                    j, par = qq // 2, qq % 2
                    for cf in cfgs:
                        (Tt, MTt, MHt, w1s_t, w1p_t, w2T_t,
                         cast, dq, aeng, minr, _pjs, _meng) = cf
                        q_lo, q_hi = _win(qq - s - 1, qq + s + 2, minr)
                        rows = q_hi - q_lo
                        uq = pst([64, 8, rows, 8])
                        nc.tensor.matmul(
                            uq[:].rearrange("p k r x -> p k (r x)"),
                            cast(w2T_t[:, par, :]),
                            cast(MHt[:, j, :, 8 * q_lo:8 * q_hi]),
                            start=True, stop=True)
                        if aeng is nc.gpsimd:
                            # GpSimd can't read PSUM: stage uq through SBUF
                            # on Act, then add on Pool (keeps DVE free)
                            uqsb = tmp.tile([64, 8, rows, 8], F32, tag="uqsb")
                            nc.scalar.copy(uqsb[:], uq[:])
                            acc_in = uqsb
                        else:
                            acc_in = uq
                        aeng.tensor_tensor(
                            out=Tt[:, 8 * qq:8 * qq + 8,
                                   1 + q_lo:1 + q_hi, 1:9],
                            in0=Tt[:, 8 * qq:8 * qq + 8,
                                   1 + q_lo:1 + q_hi, 1:9],
                            in1=acc_in[:],
                            op=ALU.add)
                if after_j is not None:
                    after_j(3)

            def w_cast(ap):
                return ap

            cfgs1 = [
                [T32, MT32, MH32, w1s0, w1p0, w2r0, w_cast,
                 sdma, nc.vector, s_min_rows, [], nc.vector],
                [T16, MT16, MH16, w1s0b, w1p0b, w2r0b, w_cast,
                 adma, nc.vector, 0, [], nc.gpsimd],
            ]
            cfgs2 = [
                [T32, MT32, MH32, w1s1, w1p1, w2r1, w_cast,
                 sdma, nc.vector, s_min_rows, [], nc.vector],
                [T16, MT16, MH16, w1s1b, w1p1b, w2r1b, w_cast,
                 adma, nc.vector, 0, [], nc.gpsimd],
            ]

            # ---- interleaved emission: fwd blocks fill PE gaps ----
            fwd_block(r0w1T, r0w2T, m1b, m2a, y1, y2)
            nc.scalar.activation(
                out=a_pad[:, 1:9, 1:9],
                in_=y2[:].rearrange("c (y x) -> c y x", y=8), func=ACTF.Relu)
            phase_mask(1, cfgs1, m1a)
            phase_conva(1, cfgs1)
            fwd_block(r1w1T, r1w2T, m2b, m3, y2, y3)
            nc.scalar.activation(out=y4[:], in_=y3[:], func=ACTF.Relu)
            yop = pst([32, 64])
            nc.tensor.matmul(yop[:], c2wT, y4[:], start=True, stop=True)
            nc.vector.tensor_scalar(out=yout[:], in0=yop[:], scalar1=b2,
                                    scalar2=None, op0=ALU.add)
            phase_mh_convb_acc(
                1, cfgs1, m1b,
                after_j=lambda j: phase_mask_j(2, cfgs2, m2a, j))

            # ================= hopfield helper =================
            def hopfield(y_ap, P, fast):
                lg = pst([64, 512])
                if fast:
                    nc.tensor.matmul(lg[:], y_ap, patTr[:],
                                     start=True, stop=True)
                else:
                    nc.tensor.matmul(lg[:], y_ap, patT, start=True, stop=True)
                ssum = tmp.tile([64, 1], F32, tag="ssum")
                # logits are tame (|lg|/sqrt(32) < 40): skip max-stabilization
                nc.scalar.activation(out=P[:], in_=lg[:], func=ACTF.Exp,
                                     bias=0.0, scale=ISQRT32, accum_out=ssum[:])
                rs = tmp.tile([64, 1], F32, tag="rs")
                nc.vector.reciprocal(rs[:], ssum[:])
                nc.scalar.mul(P[:], P[:], rs[:])
                yq = pse.tile([32, 64], F32, tag="et8", name=f"yq{_ps_n[0]}")
                for qc in range(4):
                    ptp = pst([128, 64])
                    pt = tmp.tile([128, 64], F32, tag="pt")
                    nc.tensor.transpose(ptp[:], P[:, 128 * qc:128 * (qc + 1)],
                                        ident)
                    if qc % 2 == 0:
                        nc.vector.tensor_copy(pt[:], ptp[:])
                    else:
                        nc.scalar.copy(pt[:], ptp[:])
                    nc.tensor.matmul(yq[:], pat[:, qc, :], pt[:],
                                     start=(qc == 0), stop=(qc == 3))
                return yq

            yq1 = hopfield(yout[:], P1, False)
            nc.vector.tensor_tensor(out=r_sb[:], in0=yout[:], in1=yq1[:],
                                    op=ALU.subtract)
            rps = pst([64, 64])
            nc.tensor.matmul(rps[:], c2w_oc, r_sb[:], start=True, stop=True)
            nc.vector.tensor_tensor(out=V[:], in0=rps[:], in1=m3[:],
                                    op=ALU.mult)
            phase_conva(2, cfgs2)

            # ================= C2 + routing + scatter =================
            # e_total[i,m] = sum_c V[c,m] * T32[c,i,m]; the prodE/et8
            # chunks are emitted inside stage-2's acc phase as each T32
            # chunk finalizes
            et8 = pse.tile([32, 8, 64], F32, tag="et8", name="et8")

            def emit_prode(j):
                for qq in (2 * j, 2 * j + 1):
                    peng = nc.vector if qq % 2 == 0 else nc.gpsimd
                    peng.tensor_tensor(
                        out=prodE[:, 8 * qq:8 * qq + 8, :]
                            .rearrange("p k (y x) -> p k y x", y=8),
                        in0=T32[:, 8 * qq:8 * qq + 8, 1:9, 1:9],
                        in1=V[:].rearrange("p (k y x) -> p k y x", k=1, y=8)
                            .broadcast_to((64, 8, 8, 8)),
                        op=ALU.mult)
                    # partition-sum lands on psum partition qq via one-hot
                    nc.tensor.matmul(
                        et8[:].rearrange("p k m -> p (k m)"),
                        oh8r[:, 32 * qq:32 * qq + 32],
                        prodE[:, 8 * qq:8 * qq + 8, :]
                            .rearrange("p k m -> p (k m)"),
                        start=(qq == 0), stop=(qq == 7))

            phase_mh_convb_acc(2, cfgs2, m2b, after_j=emit_prode)
            nc.vector.tensor_reduce(out=mn8[:], in_=et8[0:8, :, :], axis=AX.X,
                                    op=ALU.min)
            nc.vector.tensor_tensor(out=ohf8[:], in0=et8[0:8, :, :],
                                    in1=mn8[:].broadcast_to((8, 8, 64)),
                                    op=ALU.is_equal)
            nc.gpsimd.tensor_copy(ohrep_b[:], ohrep[:])
            # scatter: prodW[c, m, i] = T16[c, i, m] * onehot[i, m], then
            # per-chunk partial i-sums Gq -> G
            repsb = big.tile([64, 8, 8, 64], F32, tag="repsb")
            for qq in range(8):
                rep = pst([64, 8, 64])
                nc.tensor.matmul(
                    rep[:], ohrep_b[:, qq, :],
                    ohf8[:].rearrange("p k m -> p (k m)"),
                    start=True, stop=True)
                nc.scalar.copy(repsb[:, qq, :, :], rep[:])
                dst = _raw_ap(prodW[:], 8 * qq, [[1, 8], [512, 8], [64, 8]])
                weng = nc.vector if qq % 2 == 0 else nc.gpsimd
                weng.tensor_tensor(
                    out=dst,
                    in0=T16[:, 8 * qq:8 * qq + 8, 1:9, 1:9],
                    in1=repsb[:, qq, :, :].rearrange(
                        "p k (y x) -> p k y x", y=8),
                    op=ALU.mult)
                if qq % 2 == 1:
                    nc.vector.tensor_reduce(
                        out=Gq[:, :, qq // 2:qq // 2 + 1],
                        in_=prodW[:, :, 8 * qq - 8:8 * qq + 8],
                        axis=AX.X, op=ALU.add)
            nc.vector.tensor_reduce(out=G[:], in_=Gq[:, :, 0:4],
                                    axis=AX.X, op=ALU.add)
            nc.vector.tensor_tensor(
                out=G[:, :, 0], in0=G[:, :, 0], in1=m3[:], op=ALU.mult)
            ymp = pst([32, 64])
            for m0 in (0, 32):
                nc.tensor.matmul(ymp[:, m0:m0 + 32], c2wT,
                                 G[:, m0:m0 + 32, 0], start=True, stop=True)
            nc.vector.tensor_copy(ymr[:], ymp[:])

            yq2 = hopfield(ymr[:], P2, True)
            nc.vector.tensor_copy(out_sb[:], yq2[:])
            sdma(out=d_out[:], in_=out_sb[:])

    nc.compile()
    return nc


def _prep_weights(inputs):
    f = np.float32
    w1 = np.asarray(inputs['conv1_w'], f)
    w1t = w1.transpose(2, 3, 1, 0).reshape(9, 64, 64)         # [tap, c, o]
    r0 = np.asarray(inputs['res0_w1'], f).transpose(2, 3, 1, 0).reshape(9, 64, 32)
    r1 = np.asarray(inputs['res1_w1'], f).transpose(2, 3, 1, 0).reshape(9, 64, 32)
    r0w2 = np.asarray(inputs['res0_w2'], f)[:, :, 0, 0].T      # [32, 64]
    r1w2 = np.asarray(inputs['res1_w2'], f)[:, :, 0, 0].T
    pats = np.asarray(inputs['patterns'], f)

    def pack_p(r):   # [128, 3, 32]: parts 0-63 taps (ky,0), 64-127 taps (ky,1)
        return np.concatenate([r[[0, 3, 6]].transpose(1, 0, 2),
                               r[[1, 4, 7]].transpose(1, 0, 2)], axis=0)

    def dup2(w2):    # [64, 64]: parity-dup rows
        return np.concatenate([w2, w2], axis=0)

    def pad96(w):    # [P, T, 32] -> [P, T, 96] cols (w | 0 | w)
        P, T, _ = w.shape
        z = np.zeros((P, T, 96), f)
        z[:, :, 0:32] = w
        z[:, :, 64:96] = w
        return np.ascontiguousarray(z)

    def w2pad(w2):   # [32, 64] -> [64(2par*h), 2(par), 64]: par-selecting
        z = np.zeros((64, 2, 64), f)
        z[0:32, 0, :] = w2
        z[32:64, 1, :] = w2
        return np.ascontiguousarray(z)

    c = np.ascontiguousarray
    # pk64 = [b1 | c2w^T | I64 | oh8], pk32 = [b2 | c2w]
    pk64 = np.concatenate([
        np.asarray(inputs['conv1_b'], f).reshape(64, 1),
        np.asarray(inputs['conv2_w'], f)[:, :, 0, 0].T,
        np.eye(64, dtype=f),
        np.broadcast_to(np.eye(8, dtype=f), (64, 8, 8)).reshape(64, 64),
    ], axis=1)
    pk32 = np.concatenate([
        np.asarray(inputs['conv2_b'], f).reshape(32, 1),
        np.asarray(inputs['conv2_w'], f)[:, :, 0, 0],
        pats.T,
    ], axis=1)
    base = {
        'w1T': c(w1t.transpose(1, 0, 2)),
        'pk64': c(pk64),
        'pk32': c(pk32),
        'r0w1T': c(r0.transpose(1, 0, 2)),
        'w1s0': pad96(r0.transpose(1, 0, 2)),
        'w1s1': pad96(r1.transpose(1, 0, 2)),
        'w1p0': pad96(pack_p(r0)),
        'w1p1': pad96(pack_p(r1)),
        'w2r0': w2pad(r0w2),
        'w2r1': w2pad(r1w2),
        'oh8r': c(np.broadcast_to(np.concatenate(
            [np.eye(8, dtype=f), np.zeros((8, 24), f)], axis=1),
            (64, 8, 32)).reshape(64, 256)),
        'r0w2T': c(dup2(r0w2)),
        'r1w1T': c(r1.transpose(1, 0, 2)),
        'r1w2T': c(dup2(r1w2)),
        'patterns': c(pats.reshape(4, 128, 32).transpose(1, 0, 2)),
        'patternsT': c(pats.T),
        'patTr': c(pats.T),
        'zer': np.zeros((64, 3200), f),
        'zerr': np.zeros((64, 6400), f),
        'zerb': np.zeros((64, 6400), __import__('ml_dtypes').bfloat16
                         if W_MODE == 'bf16' else f),
        'ohrep': c(np.broadcast_to(np.eye(8, dtype=f)[:, :, None], (8, 8, 64))),
    }
    return base


def make_in_maps(inputs):
    x = np.asarray(inputs['x'], np.float32)
    base = _prep_weights(inputs)
    return [dict(base, x=np.ascontiguousarray(x[b].reshape(64, 64)))
            for b in range(8)]


def kernel(**inputs):
    _lazy_imports()
    from concourse.bass_utils import run_bass_kernel_spmd
    if 'nc' not in _CACHE:
        _CACHE['nc'] = build_nc()
    nc = _CACHE['nc']
    in_maps = make_in_maps(inputs)
    res = run_bass_kernel_spmd(nc, in_maps, list(range(8)))
    _CACHE['last_result'] = res
    out = np.stack([res.results[b]['out'].reshape(32, 8, 8) for b in range(8)])
    return out.astype(np.float32)


# revision 62
# speedup vs baseline: 1.1023x; 1.0146x over previous
"""Trainium2 Bass kernel for nn_Block2_87144886436578.

Reformulation: the reference materializes per-sample jacobians
J[o,m,c,i] = d propagate(x)[o,m] / d x[c,i] but only ever uses two
contractions of J:
  S[o,m,i]  = sum_c J[o,m,c,i]          (-> e_total -> argmin routing)
  Wt[o,m,i] = sum_c x[c,i] J[o,m,c,i]   (-> routed scatter y_masked)
Both are forward-mode JVPs whose input tangents live on a single pixel i:
  v_i = ones over channels at pixel i,  w_i = x[:, i] at pixel i.
So per sample we propagate 2x64 tangents through the ReLU-linearized conv
stack (masks from one forward pass). Batch is data-parallel: sample b ->
core b (8 cores).

Dtypes: S (v-tangent) half runs its convs in f32r (1 cycle/row on PE for
N>=256 vs 4 for fp32); the Wt half's conv inputs are bf16.

Support restriction: tangent i's support after stage s is a (2s+3)-row
window around its row iy, so every mask / conv / accumulate only touches
that window (windows only grow stage to stage, so stale rows outside a
window are always zero).

Emission order interleaves the forward pass with the tangent stages so
the in-order PE queue never stalls on the forward pass's serial
dependency chain.

Layout per half: tangents T [64 part(ch), 64 kk, 10, 10] zero-padded
frames; masked tangents MT [128, ...] where partitions 64-127 hold a
+1-column pre-shift of 0-63 (produced by a flat DMA copy at offset +1),
enabling tap-pair K=128 packing of the 3x3 convs: 6 PE streams instead
of 9. e_total is accumulated into an [8, 8, 64] PSUM tile via one-hot
column lhsT so the argmin reduction runs on 8 partitions.
"""
import os
import numpy as np

F32 = None  # set in _lazy_imports
_CACHE = {}

# S-half conv dtype: "f32r" (4x faster on PE) or "f32" (exact fallback).
S_MODE = os.environ.get('BASS_S_MODE', 'f32r')
# Wt-half conv-input dtype: "bf16" (4x faster) or "f32".
W_MODE = os.environ.get('BASS_W_MODE', 'bf16')


def _lazy_imports():
    global bacc, bass, tile, mybir, F32, BF16, F32R, AX, ALU, ACTF
    import concourse.bacc as bacc
    import concourse.bass as bass
    import concourse.tile as tile
    import concourse.mybir as mybir
    F32 = mybir.dt.float32
    BF16 = mybir.dt.bfloat16
    F32R = mybir.dt.float32r
    AX = mybir.AxisListType
    ALU = mybir.AluOpType
    ACTF = mybir.ActivationFunctionType


ISQRT32 = 0.17677669529663687  # 1/sqrt(32)


def _raw_ap(t_ap, extra_offset, dims):
    """AP on t_ap's tensor: keep partition dim, replace free dims."""
    return bass.AP(tensor=t_ap.tensor, offset=t_ap.offset + extra_offset,
                   ap=[list(t_ap.ap[0])] + [list(d) for d in dims])


def _win(lo, hi, min_rows=0):
    lo, hi = max(0, lo), min(8, hi)
    while hi - lo < min_rows:
        if hi < 8:
            hi += 1
        else:
            lo -= 1
    return lo, hi


def build_nc():
    _lazy_imports()
    nc = bacc.Bacc("TRN2", target_bir_lowering=False, debug=True)

    # f32r-consumed tensors are declared float32r end-to-end (the BIR
    # verifier requires every producer of f32r-matmul operands to round
    # to f32r); the forward pass keeps separate fp32 weight copies.
    SDT = F32R if S_MODE == 'f32r' else F32
    s_min_rows = 4 if S_MODE == 'f32r' else 0
    WDT = {'bf16': BF16, 'f32': F32}[W_MODE]

    # ---- DRAM I/O (per-core; weights replicated across cores) ----
    d_x = nc.dram_tensor("x", [64, 64], F32, kind="ExternalInput")
    d_w1T = nc.dram_tensor("w1T", [64, 9, 64], F32, kind="ExternalInput")
    d_pk64 = nc.dram_tensor("pk64", [64, 161], F32, kind="ExternalInput")
    d_pk32 = nc.dram_tensor("pk32", [32, 577], F32, kind="ExternalInput")
    d_r0w1T = nc.dram_tensor("r0w1T", [64, 9, 32], F32, kind="ExternalInput")
    d_r0w2T = nc.dram_tensor("r0w2T", [64, 64], F32, kind="ExternalInput")
    d_r1w1T = nc.dram_tensor("r1w1T", [64, 9, 32], F32, kind="ExternalInput")
    d_r1w2T = nc.dram_tensor("r1w2T", [64, 64], F32, kind="ExternalInput")
    d_w1s0 = nc.dram_tensor("w1s0", [64, 9, 96], SDT, kind="ExternalInput")
    d_w1s1 = nc.dram_tensor("w1s1", [64, 9, 96], SDT, kind="ExternalInput")
    d_w1p0 = nc.dram_tensor("w1p0", [128, 3, 96], SDT, kind="ExternalInput")
    d_w1p1 = nc.dram_tensor("w1p1", [128, 3, 96], SDT, kind="ExternalInput")
    d_w2r0 = nc.dram_tensor("w2r0", [64, 2, 64], SDT, kind="ExternalInput")
    d_w2r1 = nc.dram_tensor("w2r1", [64, 2, 64], SDT, kind="ExternalInput")
    d_oh8r = nc.dram_tensor("oh8r", [64, 256], SDT, kind="ExternalInput")
    d_pat = nc.dram_tensor("patterns", [128, 4, 32], F32, kind="ExternalInput")
    d_patTr = nc.dram_tensor("patTr", [32, 512], SDT, kind="ExternalInput")
    d_ohrep = nc.dram_tensor("ohrep", [8, 8, 64], F32, kind="ExternalInput")
    d_zer = nc.dram_tensor("zer", [64, 3200], F32, kind="ExternalInput")
    d_zerr = nc.dram_tensor("zerr", [64, 6400], SDT, kind="ExternalInput")
    d_zerb = nc.dram_tensor("zerb", [64, 6400], WDT, kind="ExternalInput")
    d_out = nc.dram_tensor("out", [32, 64], F32, kind="ExternalOutput")

    with tile.TileContext(nc) as tc:
        with (
            tc.tile_pool(name="big", bufs=1) as big,
            tc.tile_pool(name="tmp", bufs=4) as tmp,
            tc.tile_pool(name="psum", bufs=3, space="PSUM") as ps,
            tc.tile_pool(name="psumj", bufs=4, space="PSUM") as psj,
            tc.tile_pool(name="psume", bufs=1, space="PSUM") as pse,
        ):
            _ps_n = [0]

            def pst(shape):
                _ps_n[0] += 1
                return ps.tile(shape, F32, tag="ps", name=f"ps{_ps_n[0]}")

            # ---- persistent SBUF ----
            T32 = big.tile([64, 64, 10, 10], F32, tag="T32")
            MT32 = big.tile([128, 64, 10, 10], SDT, tag="MT32")
            MH32 = big.tile([64, 4, 8, 64], SDT, tag="MH32")  # [2par*h, j, kk8, pix]
            T16 = big.tile([64, 64, 10, 10], F32, tag="T16")
            MT16 = big.tile([128, 64, 10, 10], WDT, tag="MT16")
            MH16 = big.tile([64, 4, 8, 64], WDT, tag="MH16")

            w1T = big.tile([64, 9, 64], F32, tag="w1T")
            pk64 = big.tile([64, 161], F32, tag="pk64")
            pk32 = big.tile([32, 577], F32, tag="pk32")
            b1 = pk64[:, 0:1]
            c2wT = pk64[:, 1:33]
            ident = pk64[:, 33:97]
            b2 = pk32[:, 0:1]
            c2w_oc = pk32[:, 1:65]
            patT = pk32[:, 65:577]
            r0w1T = big.tile([64, 9, 32], F32, tag="r0w1T")
            r1w1T = big.tile([64, 9, 32], F32, tag="r1w1T")
            r0w2T = big.tile([64, 64], F32, tag="r0w2T")   # parity-dup rows
            r1w2T = big.tile([64, 64], F32, tag="r1w2T")
            # conv weights in par-padded layouts: singles [64, 9, 96]
            # (cols w|0|w, par slice at 32*par), packed [128, 3, 96],
            # w2 [64(2par*h), 2(par), 64] -- everything partition-base 0
            w1s0 = big.tile([64, 9, 96], SDT, tag="w1s0")
            w1s1 = big.tile([64, 9, 96], SDT, tag="w1s1")
            w1p0 = big.tile([128, 3, 96], SDT, tag="w1p0")
            w1p1 = big.tile([128, 3, 96], SDT, tag="w1p1")
            w2r0 = big.tile([64, 2, 64], SDT, tag="w2r0")
            w2r1 = big.tile([64, 2, 64], SDT, tag="w2r1")
            oh8r = big.tile([64, 256], SDT, tag="oh8r")
            w1s0b = big.tile([64, 9, 96], WDT, tag="w1s0b")
            w1s1b = big.tile([64, 9, 96], WDT, tag="w1s1b")
            w1p0b = big.tile([128, 3, 96], WDT, tag="w1p0b")
            w1p1b = big.tile([128, 3, 96], WDT, tag="w1p1b")
            w2r0b = big.tile([64, 2, 64], WDT, tag="w2r0b")
            w2r1b = big.tile([64, 2, 64], WDT, tag="w2r1b")
            pat = big.tile([128, 4, 32], F32, tag="pat")
            ohrep = big.tile([8, 8, 64], F32, tag="ohrep")
            ohrep_b = big.tile([8, 8, 64], BF16, tag="ohrep_b")
            ones64 = big.tile([64, 64], F32, tag="ones64")
            patTr = big.tile([32, 512], SDT, tag="patTr")
            ymr = big.tile([32, 64], SDT, tag="ymr")
            ohf8 = big.tile([8, 8, 64], BF16, tag="ohf8")
            mn8 = big.tile([8, 8, 1], F32, tag="mn8")

            x_pad = big.tile([64, 10, 10], F32, tag="x_pad")
            a_pad = big.tile([64, 10, 10], F32, tag="a_pad")
            m1a = big.tile([64, 64], F32, tag="m1a")
            m2a = big.tile([64, 64], F32, tag="m2a")
            m3 = big.tile([64, 64], F32, tag="m3")
            m1b = big.tile([64, 64], F32, tag="m1b")   # parity-dup at +32
            m2b = big.tile([64, 64], F32, tag="m2b")
            y1 = big.tile([64, 64], F32, tag="y1")
            y2 = big.tile([64, 64], F32, tag="y2")
            y3 = big.tile([64, 64], F32, tag="y3")
            y4 = big.tile([64, 64], F32, tag="y4")
            yout = big.tile([32, 64], F32, tag="yout")
            r_sb = big.tile([32, 64], F32, tag="r_sb")
            V = big.tile([64, 64], F32, tag="V")       # (c2w^T r) * m3
            P1 = big.tile([64, 512], F32, tag="P1")
            P2 = big.tile([64, 512], F32, tag="P2")
            ym = big.tile([32, 64, 1], F32, tag="ym")
            Gq = big.tile([64, 64, 8], F32, tag="Gq")
            G = big.tile([64, 64, 1], F32, tag="G")
            out_sb = big.tile([32, 64], F32, tag="out_sb")
            prodE = big.tile([64, 64, 64], SDT, tag="prodE")
            prodW = big.tile([64, 64, 64], F32, tag="prodW")

            # ---- loads: early-needed first per queue; Pool does no DMA ----
            sdma = nc.sync.dma_start
            adma = nc.scalar.dma_start
            sdma(out=x_pad[:, 1:9, 1:9],
                 in_=d_x[:].rearrange("c (y x) -> c y x", y=8))
            sdma(out=w1T[:], in_=d_w1T[:])
            sdma(out=_raw_ap(MT32[0:64, :, :, :], 0, [[1, 6400]]),
                 in_=d_zerr[:])
            sdma(out=pk64[:], in_=d_pk64[:])
            sdma(out=r0w1T[:], in_=d_r0w1T[:])
            sdma(out=w1s0[:], in_=d_w1s0[:])
            sdma(out=w1p0[:], in_=d_w1p0[:])
            sdma(out=_raw_ap(MT16[0:64, :, :, :], 0, [[1, 6400]]),
                 in_=d_zerb[:])
            sdma(out=r0w2T[:], in_=d_r0w2T[:])
            sdma(out=w2r0[:], in_=d_w2r0[:])
            sdma(out=pk32[:], in_=d_pk32[:])
            sdma(out=r1w1T[:], in_=d_r1w1T[:])
            sdma(out=w1s1[:], in_=d_w1s1[:])
            sdma(out=w1p1[:], in_=d_w1p1[:])
            sdma(out=r1w2T[:], in_=d_r1w2T[:])
            sdma(out=w2r1[:], in_=d_w2r1[:])
            sdma(out=oh8r[:], in_=d_oh8r[:])
            sdma(out=pat[:], in_=d_pat[:])
            sdma(out=ohrep[:], in_=d_ohrep[:])
            sdma(out=patTr[:], in_=d_patTr[:])

            # ---- memsets: T frames zeroed early on Pool+DVE; Act's queue
            # stays clear for the forward-pass relus ----
            nc.vector.memset(x_pad[:, :, 0:1], 0.0)
            nc.vector.memset(x_pad[:, :, 9:10], 0.0)
            nc.vector.memset(x_pad[:, 0, 1:9], 0.0)
            nc.vector.memset(x_pad[:, 9, 1:9], 0.0)
            nc.vector.memset(a_pad[:], 0.0)
            nc.vector.memset(ones64[:], 1.0)
            nc.gpsimd.memset(T32[:, 0:32, :, :], 0.0)
            nc.scalar.memzero(T32[:, 32:64, :, :])
            nc.gpsimd.memset(T16[:, 0:32, :, :], 0.0)
            nc.vector.memset(T16[:, 32:64, :, :], 0.0)
            # MT lower borders (upper halves are rewritten by the shift-copy)
            nc.vector.tensor_copy(w1s0b[:], w1s0[:])
            nc.vector.tensor_copy(w1p0b[:], w1p0[:])
            nc.vector.tensor_copy(w2r0b[:], w2r0[:])
            nc.gpsimd.tensor_copy(w1s1b[:], w1s1[:])
            nc.gpsimd.tensor_copy(w1p1b[:], w1p1[:])
            nc.gpsimd.tensor_copy(w2r1b[:], w2r1[:])

            TAPS = [(ky, kx) for ky in range(3) for kx in range(3)]

            def conv9(out_ps, wT_d, src_pad, M):
                for t, (ky, kx) in enumerate(TAPS):
                    nc.tensor.matmul(
                        out_ps, wT_d[:, t, :M],
                        src_pad[:, ky:ky + 8, kx:kx + 8],
                        start=(t == 0), stop=(t == 8))

            # ================= tangent init =================
            # T[p, kk=(iy,ix), iy+ky, ix+kx] = VW[p, (2-ky,2-kx), kk],
            # scattered straight from PSUM. Also warms up the PE pstate
            # before the forward pass's serial chain.
            for ky in range(3):
                vw3v = pst([64, 3, 64])
                vw3q = pst([64, 3, 64])
                for kx in range(3):
                    t_src = (2 - ky) * 3 + (2 - kx)
                    nc.tensor.matmul(vw3v[:, kx, :], w1T[:, t_src, :],
                                     ones64[:], start=True, stop=True)
                    nc.tensor.matmul(vw3q[:, kx, :], w1T[:, t_src, :],
                                     x_pad[:, 1:9, 1:9],
                                     start=True, stop=True)
                nc.vector.tensor_copy(
                    _raw_ap(T32[:], ky * 10, [[1, 3], [810, 8], [101, 8]]),
                    _raw_ap(vw3v[:], 0, [[64, 3], [8, 8], [1, 8]]))
                nc.vector.tensor_copy(
                    _raw_ap(T16[:], ky * 10, [[1, 3], [810, 8], [101, 8]]),
                    _raw_ap(vw3q[:], 0, [[64, 3], [8, 8], [1, 8]]))

            # ================= forward head =================
            y1p = pst([64, 64])
            conv9(y1p[:], w1T, x_pad, 64)
            nc.vector.tensor_scalar(out=y1[:], in0=y1p[:], scalar1=b1,
                                    scalar2=None, op0=ALU.add)
            nc.vector.tensor_scalar(out=m1a[:], in0=y1[:], scalar1=0.0,
                                    scalar2=None, op0=ALU.is_gt)
            nc.scalar.activation(
                out=a_pad[:, 1:9, 1:9],
                in_=y1[:].rearrange("c (y x) -> c y x", y=8), func=ACTF.Relu)

            def fwd_block(w1T_d, w2T_d, mb, ma_next, y_in, y_out):
                hp = pst([32, 64])
                conv9(hp[:], w1T_d, a_pad, 32)
                nc.vector.tensor_scalar(out=mb[0:32, :], in0=hp[:], scalar1=0.0,
                                        scalar2=None, op0=ALU.is_gt)
                sdma(out=mb[32:64, :], in_=mb[0:32, :])
                bh = tmp.tile([32, 64], F32, tag="bh")
                nc.vector.tensor_scalar_max(bh[:], hp[:], 0.0)
                up = pst([64, 64])
                nc.tensor.matmul(up[:], w2T_d[0:32, 0:64], bh[:],
                                 start=True, stop=True)
                nc.vector.tensor_tensor(out=y_out[:], in0=y_in[:], in1=up[:],
                                        op=ALU.add)
                nc.vector.tensor_scalar(out=ma_next[:], in0=y_out[:],
                                        scalar1=0.0, scalar2=None, op0=ALU.is_gt)

            # ================= tangent stage phases =================
            # cfg = (Tt, MTt, MHt, w1s, w1p, w2T, cast, dma_q, acc_eng, minr)
            def phase_mask_j(s, cfgs, ma, j):
                    for cf in cfgs:
                        Tt, MTt, dq, meng = cf[0], cf[1], cf[7], cf[11]
                        m_lo, m_hi = _win(2 * j - s, 2 * j + s + 2)
                        meng.tensor_tensor(
                            out=MTt[0:64, 16 * j:16 * j + 16,
                                    1 + m_lo:1 + m_hi, 1:9],
                            in0=Tt[:, 16 * j:16 * j + 16,
                                   1 + m_lo:1 + m_hi, 1:9],
                            in1=ma[:, 8 * m_lo:8 * m_hi].rearrange(
                                "p (k y x) -> p k y x", k=1, y=m_hi - m_lo)
                                .broadcast_to((64, 16, m_hi - m_lo, 8)),
                            op=ALU.mult)
                        # upper half = +1-flat-shift of the lower via DMA
                        dq(out=_raw_ap(MTt[64:128, :, :, :], 1600 * j,
                                       [[1, 1599]]),
                           in_=_raw_ap(MTt[0:64, :, :, :], 1600 * j + 1,
                                       [[1, 1599]]))

            def phase_mask(s, cfgs, ma):
                for j in range(4):
                    phase_mask_j(s, cfgs, ma, j)

            def phase_conva(s, cfgs):
                for j in range(4):
                    for cf in cfgs:
                        MTt, w1s_t, w1p_t, cast, minr = (
                            cf[1], cf[3], cf[4], cf[6], cf[9])
                        o_lo, o_hi = _win(2 * j - s - 1, 2 * j + s + 3, minr)
                        rows = o_hi - o_lo
                        _ps_n[0] += 1
                        pj = psj.tile([64, 8, rows, 8], F32, tag="pj",
                                      name=f"pj{_ps_n[0]}")
                        for par in range(2):
                            qq = 2 * j + par
                            # 3 single (taps (ky,2), K=64) + 3 packed
                            # (taps (ky,0)+(ky,1), K=128) streams; the
                            # par-padded lhsT slice routes par outputs to
                            # partition halves of one accumulation region
                            for ky in range(3):
                                nc.tensor.matmul(
                                    pj[:],
                                    cast(w1s_t[:, 3 * ky + 2,
                                               32 * par:32 * par + 64]),
                                    cast(MTt[0:64, 8 * qq:8 * qq + 8,
                                             ky + o_lo:ky + o_hi, 2:10]),
                                    start=(par == 0 and ky == 0), stop=False)
                                nc.tensor.matmul(
                                    pj[:],
                                    cast(w1p_t[:, ky,
                                               32 * par:32 * par + 64]),
                                    cast(MTt[0:128, 8 * qq:8 * qq + 8,
                                             ky + o_lo:ky + o_hi, 0:8]),
                                    start=False, stop=(par == 1 and ky == 2))
                        cf[10].append(pj)

            def phase_mh_convb_acc(s, cfgs, mb, after_j=None):
                # in stage 2 the W half's PSUM-reading elementwise ops move
                # off DVE (Act stages PSUM->SBUF, Pool computes) so the
                # S-half routing chain has DVE to itself in the tail
                def offl(cf):
                    return s == 2 and cf[7] is adma
                for j in range(4):
                    for cf in cfgs:
                        MHt, minr = cf[2], cf[9]
                        o_lo, o_hi = _win(2 * j - s - 1, 2 * j + s + 3, minr)
                        rows = o_hi - o_lo
                        pj = cf[10][j]
                        nc.vector.tensor_tensor(
                            out=MHt[:, j, :, 8 * o_lo:8 * o_hi],
                            in0=pj[:].rearrange("p k r x -> p k (r x)"),
                            in1=mb[:, 8 * o_lo:8 * o_hi].rearrange(
                                "p (k m) -> p k m", k=1)
                                .broadcast_to((64, 8, 8 * rows)),
                            op=ALU.mult)
                for qq in range(8):
                    if after_j is not None and qq >= 2 and qq % 2 == 0:
                        after_j(qq // 2 - 1)
                    j, par = qq // 2, qq % 2
                    for cf in cfgs:
                        (Tt, MTt, MHt, w1s_t, w1p_t, w2T_t,
                         cast, dq, aeng, minr, _pjs, _meng) = cf
                        q_lo, q_hi = _win(qq - s - 1, qq + s + 2, minr)
                        rows = q_hi - q_lo
                        uq = pst([64, 8, rows, 8])
                        nc.tensor.matmul(
                            uq[:].rearrange("p k r x -> p k (r x)"),
                            cast(w2T_t[:, par, :]),
                            cast(MHt[:, j, :, 8 * q_lo:8 * q_hi]),
                            start=True, stop=True)
                        acc_in = uq
                        aeng.tensor_tensor(
                            out=Tt[:, 8 * qq:8 * qq + 8,
                                   1 + q_lo:1 + q_hi, 1:9],
                            in0=Tt[:, 8 * qq:8 * qq + 8,
                                   1 + q_lo:1 + q_hi, 1:9],
                            in1=acc_in[:],
                            op=ALU.add)
                if after_j is not None:
                    after_j(3)

            def w_cast(ap):
                return ap

            cfgs1 = [
                [T32, MT32, MH32, w1s0, w1p0, w2r0, w_cast,
                 sdma, nc.vector, s_min_rows, [], nc.vector],
                [T16, MT16, MH16, w1s0b, w1p0b, w2r0b, w_cast,
                 adma, nc.vector, 0, [], nc.gpsimd],
            ]
            cfgs2 = [
                [T32, MT32, MH32, w1s1, w1p1, w2r1, w_cast,
                 sdma, nc.vector, s_min_rows, [], nc.vector],
                [T16, MT16, MH16, w1s1b, w1p1b, w2r1b, w_cast,
                 adma, nc.vector, 0, [], nc.gpsimd],
            ]

            # ---- interleaved emission: fwd blocks fill PE gaps ----
            fwd_block(r0w1T, r0w2T, m1b, m2a, y1, y2)
            nc.scalar.activation(
                out=a_pad[:, 1:9, 1:9],
                in_=y2[:].rearrange("c (y x) -> c y x", y=8), func=ACTF.Relu)
            phase_mask(1, cfgs1, m1a)
            phase_conva(1, cfgs1)
            fwd_block(r1w1T, r1w2T, m2b, m3, y2, y3)
            nc.scalar.activation(out=y4[:], in_=y3[:], func=ACTF.Relu)
            yop = pst([32, 64])
            nc.tensor.matmul(yop[:], c2wT, y4[:], start=True, stop=True)
            nc.vector.tensor_scalar(out=yout[:], in0=yop[:], scalar1=b2,
                                    scalar2=None, op0=ALU.add)
            phase_mh_convb_acc(
                1, cfgs1, m1b,
                after_j=lambda j: phase_mask_j(2, cfgs2, m2a, j))

            # ================= hopfield helper =================
            def hopfield(y_ap, P, fast):
                lg = pst([64, 512])
                if fast:
                    nc.tensor.matmul(lg[:], y_ap, patTr[:],
                                     start=True, stop=True)
                else:
                    nc.tensor.matmul(lg[:], y_ap, patT, start=True, stop=True)
                ssum = tmp.tile([64, 1], F32, tag="ssum")
                # logits are tame (|lg|/sqrt(32) < 40): skip max-stabilization
                nc.scalar.activation(out=P[:], in_=lg[:], func=ACTF.Exp,
                                     bias=0.0, scale=ISQRT32, accum_out=ssum[:])
                rs = tmp.tile([64, 1], F32, tag="rs")
                nc.vector.reciprocal(rs[:], ssum[:])
                nc.scalar.mul(P[:], P[:], rs[:])
                yq = pse.tile([32, 64], F32, tag="et8", name=f"yq{_ps_n[0]}")
                for qc in range(4):
                    ptp = pst([128, 64])
                    pt = tmp.tile([128, 64], F32, tag="pt")
                    nc.tensor.transpose(ptp[:], P[:, 128 * qc:128 * (qc + 1)],
                                        ident)
                    if qc % 2 == 0:
                        nc.vector.tensor_copy(pt[:], ptp[:])
                    else:
                        nc.scalar.copy(pt[:], ptp[:])
                    nc.tensor.matmul(yq[:], pat[:, qc, :], pt[:],
                                     start=(qc == 0), stop=(qc == 3))
                return yq

            yq1 = hopfield(yout[:], P1, False)
            nc.vector.tensor_tensor(out=r_sb[:], in0=yout[:], in1=yq1[:],
                                    op=ALU.subtract)
            rps = pst([64, 64])
            nc.tensor.matmul(rps[:], c2w_oc, r_sb[:], start=True, stop=True)
            nc.vector.tensor_tensor(out=V[:], in0=rps[:], in1=m3[:],
                                    op=ALU.mult)
            phase_conva(2, cfgs2)

            # ================= C2 + routing + scatter =================
            # e_total[i,m] = sum_c V[c,m] * T32[c,i,m]; the prodE/et8
            # chunks are emitted inside stage-2's acc phase as each T32
            # chunk finalizes
            et8 = pse.tile([32, 8, 64], F32, tag="et8", name="et8")

            def emit_prode(j):
                for qq in (2 * j, 2 * j + 1):
                    peng = nc.vector if qq % 2 == 0 else nc.gpsimd
                    peng.tensor_tensor(
                        out=prodE[:, 8 * qq:8 * qq + 8, :]
                            .rearrange("p k (y x) -> p k y x", y=8),
                        in0=T32[:, 8 * qq:8 * qq + 8, 1:9, 1:9],
                        in1=V[:].rearrange("p (k y x) -> p k y x", k=1, y=8)
                            .broadcast_to((64, 8, 8, 8)),
                        op=ALU.mult)
                    # partition-sum lands on psum partition qq via one-hot
                    nc.tensor.matmul(
                        et8[:].rearrange("p k m -> p (k m)"),
                        oh8r[:, 32 * qq:32 * qq + 32],
                        prodE[:, 8 * qq:8 * qq + 8, :]
                            .rearrange("p k m -> p (k m)"),
                        start=(qq == 0), stop=(qq == 7))

            phase_mh_convb_acc(2, cfgs2, m2b, after_j=emit_prode)
            nc.vector.tensor_reduce(out=mn8[:], in_=et8[0:8, :, :], axis=AX.X,
                                    op=ALU.min)
            nc.vector.tensor_tensor(out=ohf8[:], in0=et8[0:8, :, :],
                                    in1=mn8[:].broadcast_to((8, 8, 64)),
                                    op=ALU.is_equal)
            nc.gpsimd.tensor_copy(ohrep_b[:], ohrep[:])
            # scatter: prodW[c, m, i] = T16[c, i, m] * onehot[i, m], then
            # per-chunk partial i-sums Gq -> G
            repsb = big.tile([64, 8, 8, 64], F32, tag="repsb")
            for qq in range(8):
                rep = pst([64, 8, 64])
                nc.tensor.matmul(
                    rep[:], ohrep_b[:, qq, :],
                    ohf8[:].rearrange("p k m -> p (k m)"),
                    start=True, stop=True)
                nc.scalar.copy(repsb[:, qq, :, :], rep[:])
                dst = _raw_ap(prodW[:], 8 * qq, [[1, 8], [512, 8], [64, 8]])
                weng = nc.vector if qq % 2 == 0 else nc.gpsimd
                weng.tensor_tensor(
                    out=dst,
                    in0=T16[:, 8 * qq:8 * qq + 8, 1:9, 1:9],
                    in1=repsb[:, qq, :, :].rearrange(
                        "p k (y x) -> p k y x", y=8),
                    op=ALU.mult)
                if qq % 2 == 1:
                    nc.vector.tensor_reduce(
                        out=Gq[:, :, qq // 2:qq // 2 + 1],
                        in_=prodW[:, :, 8 * qq - 8:8 * qq + 8],
                        axis=AX.X, op=ALU.add)
            nc.vector.tensor_reduce(out=G[:], in_=Gq[:, :, 0:4],
                                    axis=AX.X, op=ALU.add)
            nc.vector.tensor_tensor(
                out=G[:, :, 0], in0=G[:, :, 0], in1=m3[:], op=ALU.mult)
            ymp = pst([32, 64])
            for m0 in (0, 32):
                nc.tensor.matmul(ymp[:, m0:m0 + 32], c2wT,
                                 G[:, m0:m0 + 32, 0], start=True, stop=True)
            nc.vector.tensor_copy(ymr[:], ymp[:])

            yq2 = hopfield(ymr[:], P2, True)
            nc.vector.tensor_copy(out_sb[:], yq2[:])
            sdma(out=d_out[:], in_=out_sb[:])

    nc.compile()
    return nc


def _prep_weights(inputs):
    f = np.float32
    w1 = np.asarray(inputs['conv1_w'], f)
    w1t = w1.transpose(2, 3, 1, 0).reshape(9, 64, 64)         # [tap, c, o]
    r0 = np.asarray(inputs['res0_w1'], f).transpose(2, 3, 1, 0).reshape(9, 64, 32)
    r1 = np.asarray(inputs['res1_w1'], f).transpose(2, 3, 1, 0).reshape(9, 64, 32)
    r0w2 = np.asarray(inputs['res0_w2'], f)[:, :, 0, 0].T      # [32, 64]
    r1w2 = np.asarray(inputs['res1_w2'], f)[:, :, 0, 0].T
    pats = np.asarray(inputs['patterns'], f)

    def pack_p(r):   # [128, 3, 32]: parts 0-63 taps (ky,0), 64-127 taps (ky,1)
        return np.concatenate([r[[0, 3, 6]].transpose(1, 0, 2),
                               r[[1, 4, 7]].transpose(1, 0, 2)], axis=0)

    def dup2(w2):    # [64, 64]: parity-dup rows
        return np.concatenate([w2, w2], axis=0)

    def pad96(w):    # [P, T, 32] -> [P, T, 96] cols (w | 0 | w)
        P, T, _ = w.shape
        z = np.zeros((P, T, 96), f)
        z[:, :, 0:32] = w
        z[:, :, 64:96] = w
        return np.ascontiguousarray(z)

    def w2pad(w2):   # [32, 64] -> [64(2par*h), 2(par), 64]: par-selecting
        z = np.zeros((64, 2, 64), f)
        z[0:32, 0, :] = w2
        z[32:64, 1, :] = w2
        return np.ascontiguousarray(z)

    c = np.ascontiguousarray
    # pk64 = [b1 | c2w^T | I64 | oh8], pk32 = [b2 | c2w]
    pk64 = np.concatenate([
        np.asarray(inputs['conv1_b'], f).reshape(64, 1),
        np.asarray(inputs['conv2_w'], f)[:, :, 0, 0].T,
        np.eye(64, dtype=f),
        np.broadcast_to(np.eye(8, dtype=f), (64, 8, 8)).reshape(64, 64),
    ], axis=1)
    pk32 = np.concatenate([
        np.asarray(inputs['conv2_b'], f).reshape(32, 1),
        np.asarray(inputs['conv2_w'], f)[:, :, 0, 0],
        pats.T,
    ], axis=1)
    base = {
        'w1T': c(w1t.transpose(1, 0, 2)),
        'pk64': c(pk64),
        'pk32': c(pk32),
        'r0w1T': c(r0.transpose(1, 0, 2)),
        'w1s0': pad96(r0.transpose(1, 0, 2)),
        'w1s1': pad96(r1.transpose(1, 0, 2)),
        'w1p0': pad96(pack_p(r0)),
        'w1p1': pad96(pack_p(r1)),
        'w2r0': w2pad(r0w2),
        'w2r1': w2pad(r1w2),
        'oh8r': c(np.broadcast_to(np.concatenate(
            [np.eye(8, dtype=f), np.zeros((8, 24), f)], axis=1),
            (64, 8, 32)).reshape(64, 256)),
        'r0w2T': c(dup2(r0w2)),
        'r1w1T': c(r1.transpose(1, 0, 2)),
        'r1w2T': c(dup2(r1w2)),
        'patterns': c(pats.reshape(4, 128, 32).transpose(1, 0, 2)),
        'patternsT': c(pats.T),
        'patTr': c(pats.T),
        'zer': np.zeros((64, 3200), f),
        'zerr': np.zeros((64, 6400), f),
        'zerb': np.zeros((64, 6400), __import__('ml_dtypes').bfloat16
                         if W_MODE == 'bf16' else f),
        'ohrep': c(np.broadcast_to(np.eye(8, dtype=f)[:, :, None], (8, 8, 64))),
    }
    return base


def make_in_maps(inputs):
    x = np.asarray(inputs['x'], np.float32)
    base = _prep_weights(inputs)
    return [dict(base, x=np.ascontiguousarray(x[b].reshape(64, 64)))
            for b in range(8)]


def kernel(**inputs):
    _lazy_imports()
    from concourse.bass_utils import run_bass_kernel_spmd
    if 'nc' not in _CACHE:
        _CACHE['nc'] = build_nc()
    nc = _CACHE['nc']
    in_maps = make_in_maps(inputs)
    res = run_bass_kernel_spmd(nc, in_maps, list(range(8)))
    _CACHE['last_result'] = res
    out = np.stack([res.results[b]['out'].reshape(32, 8, 8) for b in range(8)])
    return out.astype(np.float32)
